# revision 10
# baseline (speedup 1.0000x reference)
"""Multi-head attention (softmax over the QUERY axis) on 8 TRN2 NeuronCores.

Sharding: 2 batches x 4 head-groups (4 heads each) -> 8 cores; each core
processes its 4 heads as two head PAIRS (p=0,1).

Per (batch b, head pair p) on a core:
    qkT = W_{q,k} @ x_b^T + b_{q,k}        [512, 2048]  (e_out on partitions)
    V   = x_b @ W_v^T + b_v                [2048, 256]  (per-pair halves)
    S'  = K Q^T  (scores TRANSPOSED)       [k, q] per head
    P'  = exp(S'/8), denom[k] = sum_q P'   (ACT accum_out or DVE tensor_reduce)
    V'  = V[k,:]/denom[k]                  (scaled per k-tile)
    out[q, d] = sum_k P'[k,q] V'[k,d]      <-- P' stationary, V' moving (N=64)
    outT = f32 PE identity-transpose of out, bf16 on psum->sbuf copy
    part = outT^T @ WoT_p                  [2048, 1024] partial per pair, bf16
Host sums the 8 bf16 partials per batch (fp32 accumulate) and adds bo.

Design notes (vs the transposed-attn.V baseline):
- attn.V in out[q,d] form halves its PE cost (matmul cost ~ moving free
  size: N=64 instead of N=q=512 per stationary), PE ~140us vs ~165us.
- ACT (exp) is the co-bottleneck; ~40% of the softmax denominators move
  to DVE tensor_reduce so ACT and DVE both land near 70us per pair.
- GPSIMD must NOT touch PSUM (BIR verifier): every psum->sbuf evacuation
  (qk bias-adds, V copies, attn.V quad drains, transpose copies, proj
  copies) is on DVE/ACT; Pool only does DMA issue, dsum, and V-scaling.
- dma_start occupies the issuing engine ~transfer-time; input DMAs are
  spread over Pool/SP/ACT, outputs ride SP; outputs are bf16.
- PSUM (8 banks): scores 2x[128,1024]f32 + quads/transposes/pair1-V
  2x[128,4,128]f32 + qkv/proj 2x[128,512]f32.
- Schedule: pair0's qkv fillers keep PE ~balanced with ACT; pair1's loop
  carries pair0's tail (last attn.V group, transposes, projection); the
  final tail pipelines quad->transpose->proj->DMA per 4-qt block.
"""

import sys

if "/opt/trn_rl_repo" not in sys.path:
    sys.path.insert(0, "/opt/trn_rl_repo")

import numpy as np
import ml_dtypes

import concourse.bass as bass
import concourse.mybir as mybir
import concourse.tile as tile
from concourse import bacc
from concourse.bass_utils import run_bass_kernel_spmd

F32 = mybir.dt.float32
BF16 = mybir.dt.bfloat16
AF = mybir.ActivationFunctionType

B, S, E, H = 2, 2048, 1024, 16
HL = 4  # heads per core
DH = 64
QK = 512
V3 = 768
NCORES = 8

ET = E // 128  # 8
ST = S // 128  # 16
SC = S // 512  # 4
KT = ST
FG = 4  # k-tiles per attn.V group
NQUAD = 4  # qt's per attn.V psum quad

LAST_RESULTS = None


def build_kernel():
    nc = bacc.Bacc("TRN2", target_bir_lowering=False, debug=False, num_devices=NCORES)

    xT = nc.dram_tensor("xT", [E, S], BF16, kind="ExternalInput")
    wT = nc.dram_tensor("wT", [E, V3], BF16, kind="ExternalInput")
    bq = nc.dram_tensor("bq", [128, 4], F32, kind="ExternalInput")
    bv = nc.dram_tensor("bv", [1, 256], BF16, kind="ExternalInput")
    woT = nc.dram_tensor("woT", [2 * 128, E], BF16, kind="ExternalInput")
    ident = nc.dram_tensor("ident", [128, 128], F32, kind="ExternalInput")
    out0 = nc.dram_tensor("out0", [S, E], BF16, kind="ExternalOutput")
    out1 = nc.dram_tensor("out1", [S, E], BF16, kind="ExternalOutput")
    outd = {0: out0, 1: out1}

    with tile.TileContext(nc) as tc:
        with (
            tc.tile_pool(name="persist", bufs=1) as persist,
            tc.tile_pool(name="smalls", bufs=4) as smalls,
            tc.tile_pool(name="expp", bufs=2 * FG) as expp,
            tc.tile_pool(name="vsp", bufs=2 * FG + 2) as vsp,
            tc.tile_pool(name="fout", bufs=4) as foutp,
            tc.tile_pool(name="mm_ps", bufs=2, space="PSUM") as mm_ps,
            tc.tile_pool(name="sp_ps", bufs=2, space="PSUM") as sp_ps,
            tc.tile_pool(name="ot_ps", bufs=2, space="PSUM") as ot_ps,
        ):
            qk_sb = persist.tile([128, 4, S], BF16, tag="qk")
            v_sb = persist.tile([128, ST, 256], F32, tag="v")
            out_sb = persist.tile([128, 2, ST, 128], F32, tag="out")
            outT_bf = persist.tile([128, 2, S], BF16, tag="outT")
            bq_sb = persist.tile([128, 4], F32, tag="bq")
            bv_sb = persist.tile([1, 256], BF16, tag="bv")
            ones_sb = persist.tile([1, 128], BF16, tag="ones")
            id_sb = persist.tile([128, 128], F32, tag="ident")
            xt_sb = persist.tile([128, ET, S], BF16, tag="xt")
            wt_sb = persist.tile([128, ET, V3], BF16, tag="wt")
            wo_sb = persist.tile([128, 2, E], BF16, tag="wo")

            nc.vector.memset(ones_sb[:], 1.0)

            def dma_xt(sc, et):
                nc.gpsimd.dma_start(
                    xt_sb[:, et, sc * 512 : (sc + 1) * 512],
                    xT[et * 128 : (et + 1) * 128, sc * 512 : (sc + 1) * 512],
                )

            # dma_start occupies the ISSUING engine for ~the transfer time,
            # so spread input DMAs across gpsimd/SP/ACT: the critical
            # wt/sc0/sc1 chunks land in parallel within ~5us. ACT's batch
            # (sc1) sits before its exp stream and finishes by ~4.5us.
            nc.sync.dma_start(bq_sb[:], bq[:])
            for et in range(ET):
                nc.gpsimd.dma_start(wt_sb[:, et, :], wT[et * 128 : (et + 1) * 128, :])
            for et in range(ET):
                nc.sync.dma_start(
                    xt_sb[:, et, 0:512], xT[et * 128 : (et + 1) * 128, 0:512]
                )
            for et in range(ET):
                nc.scalar.dma_start(
                    xt_sb[:, et, 512:1024], xT[et * 128 : (et + 1) * 128, 512:1024]
                )
            for et in range(4):
                nc.scalar.dma_start(
                    xt_sb[:, et, 1024:1536], xT[et * 128 : (et + 1) * 128, 1024:1536]
                )
            for et in range(ET):
                nc.sync.dma_start(
                    xt_sb[:, et, 1536:2048], xT[et * 128 : (et + 1) * 128, 1536:2048]
                )
            nc.sync.dma_start(bv_sb[:], bv[:])
            nc.sync.dma_start(id_sb[:], ident[:])
            for p in range(2):
                nc.sync.dma_start(wo_sb[:, p, :], woT[p * 128 : (p + 1) * 128, :])

            # ---- qkv projection group emitters -------------------------
            # (GPSIMD cannot touch PSUM on HW: bias-adds/copies go to DVE)
            def emit_qk_group(eo, sc, bias_dve=True):
                pt = mm_ps.tile([128, 512], F32, tag="mmps")
                for et in range(ET):
                    nc.tensor.matmul(
                        pt[:],
                        wt_sb[:, et, eo * 128 : (eo + 1) * 128],
                        xt_sb[:, et, sc * 512 : (sc + 1) * 512],
                        start=(et == 0),
                        stop=(et == ET - 1),
                    )
                nc.vector.tensor_scalar_add(
                    qk_sb[:, eo, sc * 512 : (sc + 1) * 512],
                    in0=pt[:],
                    scalar1=bq_sb[:, eo : eo + 1],
                )

            def emit_v_group(st, p):
                # half V-projection: this pair's 128 v-dims only. pair1's
                # groups borrow the otps psum tag (mm is busy with pair0 proj)
                v0 = QK + p * 128
                if p == 1:
                    ptt = ot_ps.tile([128, NQUAD, 128], F32, tag="otps")
                    pt = ptt[:, 0, :]
                else:
                    ptt = mm_ps.tile([128, 512], F32, tag="mmps")
                    pt = ptt[:, 0:128]
                for et in range(ET):
                    nc.tensor.matmul(
                        pt,
                        xt_sb[:, et, st * 128 : (st + 1) * 128],
                        wt_sb[:, et, v0 : v0 + 128],
                        start=(et == 0),
                        stop=False,
                    )
                nc.tensor.matmul(  # + ones^T bv (bias row)
                    pt,
                    ones_sb[0:1, :],
                    bv_sb[0:1, p * 128 : (p + 1) * 128],
                    start=False,
                    stop=True,
                )
                nc.vector.tensor_copy(v_sb[:, st, p * 128 : (p + 1) * 128], pt)

            def qg(eo, sc):
                return lambda: emit_qk_group(eo, sc)

            def vg(st, p=0):
                return lambda: emit_v_group(st, p)

            # ---- attn.V quad (4 qt's of one group) ---------------------
            GROUPS = [(0, 4), (4, 8), (8, 12), (12, 16)]

            def emit_quad(p, g, o, exs, vss):
                k0, k1 = GROUPS[g]
                ot = ot_ps.tile([128, NQUAD, 128], F32, tag="otps")
                for qi in range(NQUAD):
                    qt = NQUAD * o + qi
                    for hh in range(2):
                        for kt in range(k0, k1):
                            nc.tensor.matmul(
                                ot[:, qi, hh * 64 : (hh + 1) * 64],
                                exs[kt][:, hh, qt * 128 : (qt + 1) * 128],
                                vss[kt][:, hh, :],
                                start=(kt == k0),
                                stop=(kt == k1 - 1),
                            )
                dst = out_sb[:, p, NQUAD * o : NQUAD * (o + 1), :]
                if g == 0:
                    nc.vector.tensor_copy(dst, ot[:])
                else:
                    nc.vector.tensor_add(dst, dst, ot[:])

            # ---- transpose block (4 qt's -> outT columns) --------------
            def emit_tblock(p, blk, tail=False):
                # f32 PE transpose straight from out_sb; the psum->sbuf copy
                # does the bf16 conversion (DVE; half on ACT at the tail)
                tp = ot_ps.tile([128, NQUAD, 128], F32, tag="otps")
                for qi in range(NQUAD):
                    nc.tensor.transpose(
                        tp[:, qi, :],
                        out_sb[:, p, NQUAD * blk + qi, :],
                        id_sb[:],
                    )
                for qi in range(NQUAD):
                    qt = NQUAD * blk + qi
                    eng = nc.scalar.copy if (tail and qi % 2 == 0) else (
                        lambda o, i: nc.vector.tensor_copy(o, i)
                    )
                    eng(
                        outT_bf[:, p, qt * 128 : (qt + 1) * 128],
                        tp[:, qi, :],
                    )

            # ---- output projection group (one s-tile, one pair) --------
            def emit_d_group(p, st, tail):
                if not tail:
                    # pair0 proj during pair1's loop: mm psum, copies on DVE
                    # (GPSIMD can't read PSUM), DMA on the idle SP
                    ot = foutp.tile([128, E], BF16, tag="fout", name=f"fo_{p}_{st}")
                    for nck in range(2):
                        pt = mm_ps.tile(
                            [128, 512], F32, tag="mmps", name=f"fp_{p}_{st}_{nck}"
                        )
                        nc.tensor.matmul(
                            pt[:],
                            outT_bf[:, p, st * 128 : (st + 1) * 128],
                            wo_sb[:, p, nck * 512 : (nck + 1) * 512],
                            start=True,
                            stop=True,
                        )
                        nc.vector.tensor_copy(
                            ot[:, nck * 512 : (nck + 1) * 512], pt[:]
                        )
                    nc.sync.dma_start(outd[p][st * 128 : (st + 1) * 128, :], ot[:])
                    return
                # tail (out1, bf16): psum alternates the freed scores slots
                # and the mm slots; both copies of a group go to ACT or DVE
                ot = foutp.tile([128, E], BF16, tag="fout", name=f"fo_{p}_{st}")
                if st % 3 == 2:
                    pts = [
                        mm_ps.tile([128, 512], F32, tag="mmps", name=f"fp_{st}_{n}")
                        for n in range(2)
                    ]
                    chunks = [pts[0][:], pts[1][:]]
                else:
                    pt = sp_ps.tile([128, 1024], F32, tag="sp", name=f"fp_{st}")
                    chunks = [pt[:, 0:512], pt[:, 512:1024]]
                for nck in range(2):
                    nc.tensor.matmul(
                        chunks[nck],
                        outT_bf[:, p, st * 128 : (st + 1) * 128],
                        wo_sb[:, p, nck * 512 : (nck + 1) * 512],
                        start=True,
                        stop=True,
                    )
                if st % 3 != 2 and st != KT - 1:
                    # contiguous psum: single wide copy; ACT takes ~7 of 11
                    # (DVE also carries the drains and transpose copies)
                    if st in (1, 6, 10, 13):
                        nc.vector.tensor_copy(ot[:], pt[:])
                    else:
                        nc.scalar.copy(ot[:], pt[:])
                else:
                    # mm-pair groups + the final group: split across engines
                    for nck in range(2):
                        eng = nc.scalar.copy if nck == 0 else (
                            lambda o, i: nc.vector.tensor_copy(o, i)
                        )
                        eng(ot[:, nck * 512 : (nck + 1) * 512], chunks[nck])
                nc.sync.dma_start(outd[p][st * 128 : (st + 1) * 128, :], ot[:])

            # ---- pre-attention: just enough for pair0 kt0 --------------
            # K-group second: its bias lands before Q-sc1's in DVE's queue,
            # so the first (512-wide) exp only waits Q-sc0 + K
            emit_qk_group(0, 0)  # Q pair0, q cols 0-511
            emit_qk_group(2, 0)  # K pair0, k tiles 0-3
            emit_qk_group(0, 1)  # Q pair0, q cols 512-1023
            # xt sc2 tail rides Pool AFTER the pre-group bias-adds so
            # those aren't queued behind the DMA issue (et0-3 went via ACT)
            for et in range(4, ET):
                nc.gpsimd.dma_start(
                    xt_sb[:, et, 1024:1536], xT[et * 128 : (et + 1) * 128, 1024:1536]
                )

            # pair0 fillers (popped per-kt by pattern below). V-projection is
            # split per pair: pair0's halves here, pair1's in its own loop.
            fillers0 = [
                vg(0), vg(1), vg(2), qg(2, 1), vg(3), vg(4), vg(5), qg(2, 2),
                vg(6), vg(7), vg(8), qg(2, 3), vg(9), vg(10), vg(11), qg(3, 0),
                vg(12), vg(13), vg(14), vg(15),
                qg(1, 0), qg(1, 1), qg(1, 2), qg(1, 3),
            ]
            fillers0.reverse()
            pops0 = [1, 3, 3, 2, 2, 2, 2, 2, 1, 1, 1, 1, 1, 1, 1, 0]  # = 24

            # pair1 in-loop fillers: its V halves + leftover K groups
            fillers1 = [
                vg(0, 1), vg(1, 1), qg(3, 1), vg(2, 1), vg(3, 1), vg(4, 1),
                qg(3, 2), vg(5, 1), vg(6, 1), vg(7, 1), qg(3, 3), vg(8, 1),
                vg(9, 1), vg(10, 1), vg(11, 1), vg(12, 1), vg(13, 1),
                vg(14, 1), vg(15, 1),
            ]
            fillers1.reverse()
            pops1 = [2, 2, 2, 2, 1, 2, 1, 2, 1, 1, 1, 1, 1, 0, 0, 0]  # = 19
            # pair0 proj schedule over pair1 kts 3..15
            proj0 = [2, 1, 1, 1, 1, 1, 1, 1, 1, 2, 1, 2, 1]  # sums to 16

            exs_p0 = vss_p0 = None
            for p in range(2):
                exs = {}
                vss = {}
                proj_done = 0
                for kt in range(KT):
                    ex = expp.tile([128, 2, S], BF16, tag="exp")
                    exs[kt] = ex
                    den = smalls.tile([128, 2, 2], F32, tag="den")
                    den2 = None
                    if p == 0 and kt == 0:
                        # fast start: first chunk split into 2x512 so the
                        # first exp fires as soon as Q-group sc0 lands
                        den2 = smalls.tile([128, 1], F32, tag="den2")
                        sp = sp_ps.tile([128, 1024], F32, tag="sp")
                        for qc in range(2):
                            nc.tensor.matmul(
                                sp[:, qc * 512 : (qc + 1) * 512],
                                qk_sb[0:64, 2, 0:128],
                                qk_sb[0:64, 0, qc * 512 : (qc + 1) * 512],
                                start=True,
                                stop=True,
                            )
                            dst = den[:, 0, 0:1] if qc == 0 else den2[:]
                            nc.scalar.activation(
                                ex[:, 0, qc * 512 : (qc + 1) * 512],
                                sp[:, qc * 512 : (qc + 1) * 512],
                                AF.Exp,
                                scale=0.125,
                                accum_out=dst,
                            )
                    for half in range(2):
                        for hh in range(2):
                            if p == 0 and kt == 0 and half == 0 and hh == 0:
                                continue  # emitted above as 2x512
                            sp = sp_ps.tile([128, 1024], F32, tag="sp")
                            for qc in range(2):
                                q0 = half * 1024 + qc * 512
                                nc.tensor.matmul(
                                    sp[:, qc * 512 : (qc + 1) * 512],
                                    qk_sb[
                                        hh * 64 : (hh + 1) * 64,
                                        2 + p,
                                        kt * 128 : (kt + 1) * 128,
                                    ],
                                    qk_sb[hh * 64 : (hh + 1) * 64, p, q0 : q0 + 512],
                                    start=True,
                                    stop=True,
                                )
                            exc = ex[:, hh, half * 1024 : (half + 1) * 1024]
                            dslice = den[:, hh, half : half + 1]
                            # offload softmax-denominator accumulation to DVE
                            # for ~40 of each pair's 64 chunks (ACT/DVE
                            # balance); keep the final k-tile on ACT so the
                            # tail's first quad isn't gated on a DVE reduce
                            # denominators via DVE tensor_scalar(mult 1.0)
                            # with fused accum: 4x DVE mode makes this ~330ns
                            # so ACT keeps only the exps. Final k-tile stays
                            # on ACT accum (shortest path into the tail).
                            offload = kt != KT - 1
                            if offload:
                                nc.scalar.activation(exc, sp[:], AF.Exp, scale=0.125)
                                nc.vector.tensor_scalar(
                                    out=exc,
                                    in0=exc,
                                    scalar1=1.0,
                                    scalar2=None,
                                    op0=mybir.AluOpType.mult,
                                    op1=mybir.AluOpType.add,
                                    accum_out=dslice,
                                )
                            else:
                                nc.scalar.activation(
                                    exc, sp[:], AF.Exp, scale=0.125, accum_out=dslice
                                )
                        if kt == 0 and half == 0 and hh == 1 and p == 0:
                            # Q cols 1024-2047 for pair0 (needed by half1);
                            # pair1's Q groups all complete during pair0.
                            emit_qk_group(0, 2)
                            emit_qk_group(0, 3)

                    # attn.V quad of the previous group (or pair0 spill)
                    if kt >= FG:
                        g, o = kt // FG - 1, kt % FG
                        emit_quad(p, g, o, exs, vss)
                    elif p == 1:
                        emit_quad(0, 3, kt, exs_p0, vss_p0)

                    # fillers / pair0-tail / proj interleaves
                    if p == 0:
                        for _ in range(pops0[kt]):
                            if fillers0:
                                fillers0.pop()()
                    else:
                        for _ in range(pops1[kt]):
                            if fillers1:
                                fillers1.pop()()
                        if 2 <= kt <= 5:
                            emit_tblock(0, kt - 2)
                        if kt >= 3:
                            for _ in range(proj0[kt - 3]):
                                if proj_done < NQUAD * (kt - 1):
                                    emit_d_group(0, proj_done, tail=False)
                                    proj_done += 1

                    # denominators -> 1/denom -> scaled V for this k-tile
                    # (combine/scale on Pool; reciprocal is DVE-only)
                    dsum = smalls.tile([128, 2], F32, tag="dsum")
                    nc.gpsimd.tensor_add(dsum[:], den[:, :, 0], den[:, :, 1])
                    if den2 is not None:
                        nc.gpsimd.tensor_add(dsum[:, 0:1], dsum[:, 0:1], den2[:])
                    rec = smalls.tile([128, 2], F32, tag="rec")
                    nc.vector.reciprocal(rec[:], dsum[:])
                    vs = vsp.tile([128, 2, DH], BF16, tag="vs")
                    vss[kt] = vs
                    for hh in range(2):
                        nc.gpsimd.tensor_scalar_mul(
                            vs[:, hh, :],
                            in0=v_sb[:, kt, (2 * p + hh) * 64 : (2 * p + hh + 1) * 64],
                            scalar1=rec[:, hh : hh + 1],
                        )

                if p == 0:
                    exs_p0, vss_p0 = exs, vss
                else:
                    # tail: stay one quad ahead so PE never idles while a
                    # block's drain/convert hop across DVE/ACT
                    emit_quad(1, 3, 0, exs, vss)
                    for o in range(NQUAD):
                        if o + 1 < NQUAD:
                            emit_quad(1, 3, o + 1, exs, vss)
                        emit_tblock(1, o, tail=True)
                        for st in range(NQUAD * o, NQUAD * (o + 1)):
                            emit_d_group(1, st, tail=True)

    nc.compile()
    return nc


def _shard_inputs(input, Wqkv, bqkv, Wo):
    """Build the 8 per-core input dicts (host-side layout/sharding)."""
    bf16 = ml_dtypes.bfloat16
    ident_f32 = np.eye(128, dtype=np.float32)
    in_maps = []
    for c in range(NCORES):
        b = c // 4
        g = c % 4
        heads = range(4 * g, 4 * g + 4)
        rows = (
            [slice(64 * h, 64 * h + 64) for h in heads]
            + [slice(E + 64 * h, E + 64 * h + 64) for h in heads]
            + [slice(2 * E + 64 * h, 2 * E + 64 * h + 64) for h in heads]
        )
        W_sel = np.concatenate([Wqkv[s] for s in rows], axis=0)  # [768, 1024]
        b_sel = np.concatenate([bqkv[s] for s in rows], axis=0)  # [768]
        in_maps.append(
            {
                "xT": np.ascontiguousarray(input[b].T).astype(bf16),
                "wT": np.ascontiguousarray(W_sel.T).astype(bf16),
                "bq": np.ascontiguousarray(b_sel[:QK].reshape(4, 128).T),
                "bv": np.ascontiguousarray(b_sel[QK:V3].reshape(1, 256)).astype(bf16),
                "woT": np.ascontiguousarray(
                    Wo[:, 4 * g * DH : 4 * (g + 1) * DH].T
                ).astype(bf16),
                "ident": ident_f32,
            }
        )
    return in_maps


def kernel(input, Wqkv, bqkv, Wo, bo, _trace=False):
    global LAST_RESULTS
    input = np.asarray(input, dtype=np.float32)
    Wqkv = np.asarray(Wqkv, dtype=np.float32)
    bqkv = np.asarray(bqkv, dtype=np.float32)
    Wo = np.asarray(Wo, dtype=np.float32)
    bo = np.asarray(bo, dtype=np.float32)

    nc = build_kernel()
    in_maps = _shard_inputs(input, Wqkv, bqkv, Wo)
    kwargs = {}
    if _trace:
        kwargs = dict(trace=True, trace_cores=[0])
    try:
        res = run_bass_kernel_spmd(nc, in_maps, core_ids=list(range(NCORES)), **kwargs)
    except ModuleNotFoundError:
        # no NTFF profiling hook in this container — run without trace
        res = run_bass_kernel_spmd(nc, in_maps, core_ids=list(range(NCORES)))
    LAST_RESULTS = res

    out = np.zeros((B, S, E), dtype=np.float32)
    for c in range(NCORES):
        out[c // 4] += res.results[c]["out0"]
        out[c // 4] += res.results[c]["out1"]
    out += bo
    return out


# revision 12
# speedup vs baseline: 1.0051x; 1.0051x over previous
"""Multi-head attention (softmax over the QUERY axis) on 8 TRN2 NeuronCores.

Sharding: 2 batches x 4 head-groups (4 heads each) -> 8 cores; each core
processes its 4 heads as two head PAIRS (p=0,1).

Per (batch b, head pair p) on a core:
    qkT = W_{q,k} @ x_b^T + b_{q,k}        [512, 2048]  (e_out on partitions)
    V   = x_b @ W_v^T + b_v                [2048, 256]  (per-pair halves)
    S'  = K Q^T  (scores TRANSPOSED)       [k, q] per head
    P'  = exp(S'/8), denom[k] = sum_q P'   (ACT accum_out or DVE tensor_reduce)
    V'  = V[k,:]/denom[k]                  (scaled per k-tile)
    out[q, d] = sum_k P'[k,q] V'[k,d]      <-- P' stationary, V' moving (N=64)
    outT = f32 PE identity-transpose of out, bf16 on psum->sbuf copy
    part = outT^T @ WoT_p                  [2048, 1024] partial per pair, bf16
Host sums the 8 bf16 partials per batch (fp32 accumulate) and adds bo.

Design notes (vs the transposed-attn.V baseline):
- attn.V in out[q,d] form halves its PE cost (matmul cost ~ moving free
  size: N=64 instead of N=q=512 per stationary), PE ~140us vs ~165us.
- The exp stream is the bottleneck and runs on ACT alone (~134us,
  gapless). Softmax denominators come from DVE tensor_scalar(mult 1.0,
  op1=add, accum_out) on the bf16 P' in SBUF, which hits the 4x DVE
  mode (~330ns per [128,1024] chunk vs 1127ns for tensor_reduce); only
  the final k-tile keeps ACT accum_out (shortest path into the tail).
- GPSIMD must NOT touch PSUM (BIR verifier): every psum->sbuf evacuation
  (qk bias-adds, V copies, attn.V quad drains, transpose copies, proj
  copies) is on DVE/ACT; Pool only does DMA issue, dsum, and V-scaling.
- dma_start occupies the issuing engine ~transfer-time; input DMAs are
  spread over Pool/SP/ACT, outputs ride SP; outputs are bf16.
- PSUM (8 banks): scores 2x[128,1024]f32 + quads/transposes/pair1-V
  2x[128,4,128]f32 + qkv/proj 2x[128,512]f32.
- Schedule: pair0's qkv fillers keep PE ~balanced with ACT; pair1's loop
  carries pair0's tail (last attn.V group, transposes, projection); the
  final tail pipelines quad->transpose->proj->DMA per 4-qt block.
"""

import sys

if "/opt/trn_rl_repo" not in sys.path:
    sys.path.insert(0, "/opt/trn_rl_repo")

import numpy as np
import ml_dtypes

import concourse.bass as bass
import concourse.mybir as mybir
import concourse.tile as tile
from concourse import bacc
from concourse.bass_utils import run_bass_kernel_spmd

F32 = mybir.dt.float32
BF16 = mybir.dt.bfloat16
AF = mybir.ActivationFunctionType

B, S, E, H = 2, 2048, 1024, 16
HL = 4  # heads per core
DH = 64
QK = 512
V3 = 768
NCORES = 8

ET = E // 128  # 8
ST = S // 128  # 16
SC = S // 512  # 4
KT = ST
FG = 4  # k-tiles per attn.V group
NQUAD = 4  # qt's per attn.V psum quad

LAST_RESULTS = None


def build_kernel():
    nc = bacc.Bacc("TRN2", target_bir_lowering=False, debug=False, num_devices=NCORES)

    xT = nc.dram_tensor("xT", [E, S], BF16, kind="ExternalInput")
    wT = nc.dram_tensor("wT", [E, V3], BF16, kind="ExternalInput")
    bq = nc.dram_tensor("bq", [128, 4], F32, kind="ExternalInput")
    bv = nc.dram_tensor("bv", [1, 256], BF16, kind="ExternalInput")
    woT = nc.dram_tensor("woT", [2 * 128, E], BF16, kind="ExternalInput")
    ident = nc.dram_tensor("ident", [128, 128], F32, kind="ExternalInput")
    out0 = nc.dram_tensor("out0", [S, E], BF16, kind="ExternalOutput")
    out1 = nc.dram_tensor("out1", [S, E], BF16, kind="ExternalOutput")
    outd = {0: out0, 1: out1}

    with tile.TileContext(nc) as tc:
        with (
            tc.tile_pool(name="persist", bufs=1) as persist,
            tc.tile_pool(name="smalls", bufs=4) as smalls,
            tc.tile_pool(name="expp", bufs=2 * FG) as expp,
            tc.tile_pool(name="vsp", bufs=2 * FG + 2) as vsp,
            tc.tile_pool(name="fout", bufs=4) as foutp,
            tc.tile_pool(name="mm_ps", bufs=2, space="PSUM") as mm_ps,
            tc.tile_pool(name="sp_ps", bufs=2, space="PSUM") as sp_ps,
            tc.tile_pool(name="ot_ps", bufs=2, space="PSUM") as ot_ps,
        ):
            qk_sb = persist.tile([128, 4, S], BF16, tag="qk")
            v_sb = persist.tile([128, ST, 256], F32, tag="v")
            out_sb = persist.tile([128, 2, ST, 128], F32, tag="out")
            outT_bf = persist.tile([128, 2, S], BF16, tag="outT")
            bq_sb = persist.tile([128, 4], F32, tag="bq")
            bv_sb = persist.tile([1, 256], BF16, tag="bv")
            ones_sb = persist.tile([1, 128], BF16, tag="ones")
            id_sb = persist.tile([128, 128], F32, tag="ident")
            xt_sb = persist.tile([128, ET, S], BF16, tag="xt")
            wt_sb = persist.tile([128, ET, V3], BF16, tag="wt")
            wo_sb = persist.tile([128, 2, E], BF16, tag="wo")

            nc.vector.memset(ones_sb[:], 1.0)

            def dma_xt(sc, et):
                nc.gpsimd.dma_start(
                    xt_sb[:, et, sc * 512 : (sc + 1) * 512],
                    xT[et * 128 : (et + 1) * 128, sc * 512 : (sc + 1) * 512],
                )

            # dma_start occupies the ISSUING engine for ~the transfer time,
            # so spread input DMAs across gpsimd/SP/ACT: the critical
            # wt/sc0/sc1 chunks land in parallel within ~5us. ACT's batch
            # (sc1) sits before its exp stream and finishes by ~4.5us.
            nc.sync.dma_start(bq_sb[:], bq[:])
            for et in range(ET):
                nc.gpsimd.dma_start(wt_sb[:, et, :], wT[et * 128 : (et + 1) * 128, :])
            for et in range(ET):
                nc.sync.dma_start(
                    xt_sb[:, et, 0:512], xT[et * 128 : (et + 1) * 128, 0:512]
                )
            for et in range(ET):
                nc.scalar.dma_start(
                    xt_sb[:, et, 512:1024], xT[et * 128 : (et + 1) * 128, 512:1024]
                )
            for et in range(5):
                nc.scalar.dma_start(
                    xt_sb[:, et, 1024:1536], xT[et * 128 : (et + 1) * 128, 1024:1536]
                )
            for et in range(5):
                nc.sync.dma_start(
                    xt_sb[:, et, 1536:2048], xT[et * 128 : (et + 1) * 128, 1536:2048]
                )
            for et in range(5, ET):
                nc.gpsimd.dma_start(
                    xt_sb[:, et, 1024:1536], xT[et * 128 : (et + 1) * 128, 1024:1536]
                )
                nc.gpsimd.dma_start(
                    xt_sb[:, et, 1536:2048], xT[et * 128 : (et + 1) * 128, 1536:2048]
                )
            nc.sync.dma_start(bv_sb[:], bv[:])
            nc.sync.dma_start(id_sb[:], ident[:])
            for p in range(2):
                nc.sync.dma_start(wo_sb[:, p, :], woT[p * 128 : (p + 1) * 128, :])

            # ---- qkv projection group emitters -------------------------
            # (GPSIMD cannot touch PSUM on HW: bias-adds/copies go to DVE)
            def emit_qk_group(eo, sc, bias_dve=True):
                pt = mm_ps.tile([128, 512], F32, tag="mmps")
                for et in range(ET):
                    nc.tensor.matmul(
                        pt[:],
                        wt_sb[:, et, eo * 128 : (eo + 1) * 128],
                        xt_sb[:, et, sc * 512 : (sc + 1) * 512],
                        start=(et == 0),
                        stop=(et == ET - 1),
                    )
                nc.vector.tensor_scalar_add(
                    qk_sb[:, eo, sc * 512 : (sc + 1) * 512],
                    in0=pt[:],
                    scalar1=bq_sb[:, eo : eo + 1],
                )

            def emit_v_group(st, p):
                # half V-projection: this pair's 128 v-dims only. pair1's
                # groups borrow the otps psum tag (mm is busy with pair0 proj)
                v0 = QK + p * 128
                if p == 1:
                    ptt = ot_ps.tile([128, NQUAD, 128], F32, tag="otps")
                    pt = ptt[:, 0, :]
                else:
                    ptt = mm_ps.tile([128, 512], F32, tag="mmps")
                    pt = ptt[:, 0:128]
                for et in range(ET):
                    nc.tensor.matmul(
                        pt,
                        xt_sb[:, et, st * 128 : (st + 1) * 128],
                        wt_sb[:, et, v0 : v0 + 128],
                        start=(et == 0),
                        stop=False,
                    )
                nc.tensor.matmul(  # + ones^T bv (bias row)
                    pt,
                    ones_sb[0:1, :],
                    bv_sb[0:1, p * 128 : (p + 1) * 128],
                    start=False,
                    stop=True,
                )
                nc.vector.tensor_copy(v_sb[:, st, p * 128 : (p + 1) * 128], pt)

            def qg(eo, sc):
                return lambda: emit_qk_group(eo, sc)

            def vg(st, p=0):
                return lambda: emit_v_group(st, p)

            # ---- attn.V quad (4 qt's of one group) ---------------------
            GROUPS = [(0, 4), (4, 8), (8, 12), (12, 16)]

            def emit_quad(p, g, o, exs, vss):
                k0, k1 = GROUPS[g]
                ot = ot_ps.tile([128, NQUAD, 128], F32, tag="otps")
                for qi in range(NQUAD):
                    qt = NQUAD * o + qi
                    for hh in range(2):
                        for kt in range(k0, k1):
                            nc.tensor.matmul(
                                ot[:, qi, hh * 64 : (hh + 1) * 64],
                                exs[kt][:, hh, qt * 128 : (qt + 1) * 128],
                                vss[kt][:, hh, :],
                                start=(kt == k0),
                                stop=(kt == k1 - 1),
                            )
                dst = out_sb[:, p, NQUAD * o : NQUAD * (o + 1), :]
                if g == 0:
                    nc.vector.tensor_copy(dst, ot[:])
                else:
                    nc.vector.tensor_add(dst, dst, ot[:])

            # ---- transpose block (4 qt's -> outT columns) --------------
            def emit_tblock(p, blk, tail=False):
                # f32 PE transpose straight from out_sb; the psum->sbuf copy
                # does the bf16 conversion (DVE; half on ACT at the tail)
                tp = ot_ps.tile([128, NQUAD, 128], F32, tag="otps")
                for qi in range(NQUAD):
                    nc.tensor.transpose(
                        tp[:, qi, :],
                        out_sb[:, p, NQUAD * blk + qi, :],
                        id_sb[:],
                    )
                for qi in range(NQUAD):
                    qt = NQUAD * blk + qi
                    eng = nc.scalar.copy if (tail and qi % 2 == 0) else (
                        lambda o, i: nc.vector.tensor_copy(o, i)
                    )
                    eng(
                        outT_bf[:, p, qt * 128 : (qt + 1) * 128],
                        tp[:, qi, :],
                    )

            # ---- output projection group (one s-tile, one pair) --------
            def emit_d_group(p, st, tail):
                if not tail:
                    # pair0 proj during pair1's loop: mm psum, copies on DVE
                    # (GPSIMD can't read PSUM), DMA on the idle SP
                    ot = foutp.tile([128, E], BF16, tag="fout", name=f"fo_{p}_{st}")
                    for nck in range(2):
                        pt = mm_ps.tile(
                            [128, 512], F32, tag="mmps", name=f"fp_{p}_{st}_{nck}"
                        )
                        nc.tensor.matmul(
                            pt[:],
                            outT_bf[:, p, st * 128 : (st + 1) * 128],
                            wo_sb[:, p, nck * 512 : (nck + 1) * 512],
                            start=True,
                            stop=True,
                        )
                        nc.vector.tensor_copy(
                            ot[:, nck * 512 : (nck + 1) * 512], pt[:]
                        )
                    nc.sync.dma_start(outd[p][st * 128 : (st + 1) * 128, :], ot[:])
                    return
                # tail (out1, bf16): psum alternates the freed scores slots
                # and the mm slots; both copies of a group go to ACT or DVE
                ot = foutp.tile([128, E], BF16, tag="fout", name=f"fo_{p}_{st}")
                if st % 3 == 2:
                    pts = [
                        mm_ps.tile([128, 512], F32, tag="mmps", name=f"fp_{st}_{n}")
                        for n in range(2)
                    ]
                    chunks = [pts[0][:], pts[1][:]]
                else:
                    pt = sp_ps.tile([128, 1024], F32, tag="sp", name=f"fp_{st}")
                    chunks = [pt[:, 0:512], pt[:, 512:1024]]
                for nck in range(2):
                    nc.tensor.matmul(
                        chunks[nck],
                        outT_bf[:, p, st * 128 : (st + 1) * 128],
                        wo_sb[:, p, nck * 512 : (nck + 1) * 512],
                        start=True,
                        stop=True,
                    )
                if st % 3 != 2 and st != KT - 1:
                    # contiguous psum: single wide copy; ACT takes ~7 of 11
                    # (DVE also carries the drains and transpose copies)
                    if st in (1, 6, 10, 13):
                        nc.vector.tensor_copy(ot[:], pt[:])
                    else:
                        nc.scalar.copy(ot[:], pt[:])
                else:
                    # mm-pair groups + the final group: split across engines
                    for nck in range(2):
                        eng = nc.scalar.copy if nck == 0 else (
                            lambda o, i: nc.vector.tensor_copy(o, i)
                        )
                        eng(ot[:, nck * 512 : (nck + 1) * 512], chunks[nck])
                nc.sync.dma_start(outd[p][st * 128 : (st + 1) * 128, :], ot[:])

            # ---- pre-attention: just enough for pair0 kt0 --------------
            # only what the FIRST 512-wide exp needs; Q-sc1 is emitted
            # between the two split chunks inside kt0
            emit_qk_group(0, 0)  # Q pair0, q cols 0-511
            emit_qk_group(2, 0)  # K pair0, k tiles 0-3


            # pair0 fillers (popped per-kt by pattern below). V-projection is
            # split per pair: pair0's halves here, pair1's in its own loop.
            fillers0 = [
                vg(0), vg(1), vg(2), qg(2, 1), vg(3), vg(4), vg(5), qg(2, 2),
                vg(6), vg(7), vg(8), qg(2, 3), vg(9), vg(10), vg(11), qg(3, 0),
                vg(12), vg(13), vg(14), vg(15),
                qg(1, 0), qg(1, 1), qg(1, 2), qg(1, 3),
            ]
            fillers0.reverse()
            pops0 = [1, 3, 3, 2, 2, 2, 2, 2, 1, 1, 1, 1, 1, 1, 1, 0]  # = 24

            # pair1 in-loop fillers: its V halves + leftover K groups
            fillers1 = [
                vg(0, 1), vg(1, 1), qg(3, 1), vg(2, 1), vg(3, 1), vg(4, 1),
                qg(3, 2), vg(5, 1), vg(6, 1), vg(7, 1), qg(3, 3), vg(8, 1),
                vg(9, 1), vg(10, 1), vg(11, 1), vg(12, 1), vg(13, 1),
                vg(14, 1), vg(15, 1),
            ]
            fillers1.reverse()
            pops1 = [2, 2, 2, 2, 1, 2, 1, 2, 1, 1, 1, 1, 1, 0, 0, 0]  # = 19
            # pair0 proj schedule over pair1 kts 3..15
            proj0 = [2, 1, 1, 1, 1, 1, 1, 1, 1, 2, 1, 2, 1]  # sums to 16

            exs_p0 = vss_p0 = None
            for p in range(2):
                exs = {}
                vss = {}
                proj_done = 0
                for kt in range(KT):
                    ex = expp.tile([128, 2, S], BF16, tag="exp")
                    exs[kt] = ex
                    den = smalls.tile([128, 2, 2], F32, tag="den")
                    den2 = None
                    if p == 0 and kt == 0:
                        # fast start: first chunk split into 2x512 so the
                        # first exp fires as soon as Q-group sc0 lands
                        den2 = smalls.tile([128, 1], F32, tag="den2")
                        sp = sp_ps.tile([128, 1024], F32, tag="sp")
                        for qc in range(2):
                            nc.tensor.matmul(
                                sp[:, qc * 512 : (qc + 1) * 512],
                                qk_sb[0:64, 2, 0:128],
                                qk_sb[0:64, 0, qc * 512 : (qc + 1) * 512],
                                start=True,
                                stop=True,
                            )
                            dst = den[:, 0, 0:1] if qc == 0 else den2[:]
                            nc.scalar.activation(
                                ex[:, 0, qc * 512 : (qc + 1) * 512],
                                sp[:, qc * 512 : (qc + 1) * 512],
                                AF.Exp,
                                scale=0.125,
                                accum_out=dst,
                            )
                            if qc == 0:
                                emit_qk_group(0, 1)  # Q cols 512-1023
                    for half in range(2):
                        for hh in range(2):
                            if p == 0 and kt == 0 and half == 0 and hh == 0:
                                continue  # emitted above as 2x512
                            sp = sp_ps.tile([128, 1024], F32, tag="sp")
                            for qc in range(2):
                                q0 = half * 1024 + qc * 512
                                nc.tensor.matmul(
                                    sp[:, qc * 512 : (qc + 1) * 512],
                                    qk_sb[
                                        hh * 64 : (hh + 1) * 64,
                                        2 + p,
                                        kt * 128 : (kt + 1) * 128,
                                    ],
                                    qk_sb[hh * 64 : (hh + 1) * 64, p, q0 : q0 + 512],
                                    start=True,
                                    stop=True,
                                )
                            exc = ex[:, hh, half * 1024 : (half + 1) * 1024]
                            dslice = den[:, hh, half : half + 1]
                            # offload softmax-denominator accumulation to DVE
                            # for ~40 of each pair's 64 chunks (ACT/DVE
                            # balance); keep the final k-tile on ACT so the
                            # tail's first quad isn't gated on a DVE reduce
                            # denominators via DVE tensor_scalar(mult 1.0)
                            # with fused accum: 4x DVE mode makes this ~330ns
                            # so ACT keeps only the exps. Final k-tile stays
                            # on ACT accum (shortest path into the tail).
                            offload = True
                            if offload:
                                nc.scalar.activation(exc, sp[:], AF.Exp, scale=0.125)
                                nc.vector.tensor_scalar(
                                    out=exc,
                                    in0=exc,
                                    scalar1=1.0,
                                    scalar2=None,
                                    op0=mybir.AluOpType.mult,
                                    op1=mybir.AluOpType.add,
                                    accum_out=dslice,
                                )
                            else:
                                nc.scalar.activation(
                                    exc, sp[:], AF.Exp, scale=0.125, accum_out=dslice
                                )
                        if kt == 0 and half == 0 and hh == 1 and p == 0:
                            # Q cols 1024-2047 for pair0 (needed by half1);
                            # pair1's Q groups all complete during pair0.
                            emit_qk_group(0, 2)
                            emit_qk_group(0, 3)

                    # attn.V quad of the previous group (or pair0 spill)
                    if kt >= FG:
                        g, o = kt // FG - 1, kt % FG
                        emit_quad(p, g, o, exs, vss)
                    elif p == 1:
                        emit_quad(0, 3, kt, exs_p0, vss_p0)

                    # fillers / pair0-tail / proj interleaves
                    if p == 0:
                        for _ in range(pops0[kt]):
                            if fillers0:
                                fillers0.pop()()
                    else:
                        for _ in range(pops1[kt]):
                            if fillers1:
                                fillers1.pop()()
                        if 2 <= kt <= 5:
                            emit_tblock(0, kt - 2)
                        if kt >= 3:
                            for _ in range(proj0[kt - 3]):
                                if proj_done < NQUAD * (kt - 1):
                                    emit_d_group(0, proj_done, tail=False)
                                    proj_done += 1

                    # denominators -> 1/denom -> scaled V for this k-tile
                    # (combine/scale on Pool; reciprocal is DVE-only)
                    dsum = smalls.tile([128, 2], F32, tag="dsum")
                    nc.gpsimd.tensor_add(dsum[:], den[:, :, 0], den[:, :, 1])
                    if den2 is not None:
                        nc.gpsimd.tensor_add(dsum[:, 0:1], dsum[:, 0:1], den2[:])
                    rec = smalls.tile([128, 2], F32, tag="rec")
                    nc.vector.reciprocal(rec[:], dsum[:])
                    vs = vsp.tile([128, 2, DH], BF16, tag="vs")
                    vss[kt] = vs
                    for hh in range(2):
                        nc.gpsimd.tensor_scalar_mul(
                            vs[:, hh, :],
                            in0=v_sb[:, kt, (2 * p + hh) * 64 : (2 * p + hh + 1) * 64],
                            scalar1=rec[:, hh : hh + 1],
                        )

                if p == 0:
                    exs_p0, vss_p0 = exs, vss
                else:
                    # tail: stay one quad ahead so PE never idles while a
                    # block's drain/convert hop across DVE/ACT
                    emit_quad(1, 3, 0, exs, vss)
                    for o in range(NQUAD):
                        if o + 1 < NQUAD:
                            emit_quad(1, 3, o + 1, exs, vss)
                        emit_tblock(1, o, tail=True)
                        for st in range(NQUAD * o, NQUAD * (o + 1)):
                            emit_d_group(1, st, tail=True)

    nc.compile()
    return nc


def _shard_inputs(input, Wqkv, bqkv, Wo):
    """Build the 8 per-core input dicts (host-side layout/sharding)."""
    bf16 = ml_dtypes.bfloat16
    ident_f32 = np.eye(128, dtype=np.float32)
    in_maps = []
    for c in range(NCORES):
        b = c // 4
        g = c % 4
        heads = range(4 * g, 4 * g + 4)
        rows = (
            [slice(64 * h, 64 * h + 64) for h in heads]
            + [slice(E + 64 * h, E + 64 * h + 64) for h in heads]
            + [slice(2 * E + 64 * h, 2 * E + 64 * h + 64) for h in heads]
        )
        W_sel = np.concatenate([Wqkv[s] for s in rows], axis=0)  # [768, 1024]
        b_sel = np.concatenate([bqkv[s] for s in rows], axis=0)  # [768]
        in_maps.append(
            {
                "xT": np.ascontiguousarray(input[b].T).astype(bf16),
                "wT": np.ascontiguousarray(W_sel.T).astype(bf16),
                "bq": np.ascontiguousarray(b_sel[:QK].reshape(4, 128).T),
                "bv": np.ascontiguousarray(b_sel[QK:V3].reshape(1, 256)).astype(bf16),
                "woT": np.ascontiguousarray(
                    Wo[:, 4 * g * DH : 4 * (g + 1) * DH].T
                ).astype(bf16),
                "ident": ident_f32,
            }
        )
    return in_maps


def kernel(input, Wqkv, bqkv, Wo, bo, _trace=False):
    global LAST_RESULTS
    input = np.asarray(input, dtype=np.float32)
    Wqkv = np.asarray(Wqkv, dtype=np.float32)
    bqkv = np.asarray(bqkv, dtype=np.float32)
    Wo = np.asarray(Wo, dtype=np.float32)
    bo = np.asarray(bo, dtype=np.float32)

    nc = build_kernel()
    in_maps = _shard_inputs(input, Wqkv, bqkv, Wo)
    kwargs = {}
    if _trace:
        kwargs = dict(trace=True, trace_cores=[0])
    try:
        res = run_bass_kernel_spmd(nc, in_maps, core_ids=list(range(NCORES)), **kwargs)
    except ModuleNotFoundError:
        # no NTFF profiling hook in this container — run without trace
        res = run_bass_kernel_spmd(nc, in_maps, core_ids=list(range(NCORES)))
    LAST_RESULTS = res

    out = np.zeros((B, S, E), dtype=np.float32)
    for c in range(NCORES):
        out[c // 4] += res.results[c]["out0"]
        out[c // 4] += res.results[c]["out1"]
    out += bo
    return out


# revision 13
# speedup vs baseline: 1.0126x; 1.0074x over previous
"""Multi-head attention (softmax over the QUERY axis) on 8 TRN2 NeuronCores.

Sharding: 2 batches x 4 head-groups (4 heads each) -> 8 cores; each core
processes its 4 heads as two head PAIRS (p=0,1).

Per (batch b, head pair p) on a core:
    qkT = W_{q,k} @ x_b^T + b_{q,k}        [512, 2048]  (e_out on partitions)
    V   = x_b @ W_v^T + b_v                [2048, 256]  (per-pair halves)
    S'  = K Q^T  (scores TRANSPOSED)       [k, q] per head
    P'  = exp(S'/8), denom[k] = sum_q P'   (ACT accum_out or DVE tensor_reduce)
    V'  = V[k,:]/denom[k]                  (scaled per k-tile)
    out[q, d] = sum_k P'[k,q] V'[k,d]      <-- P' stationary, V' moving (N=64)
    outT = f32 PE identity-transpose of out, bf16 on psum->sbuf copy
    part = outT^T @ WoT_p                  [2048, 1024] partial per pair, bf16
Host sums the 8 bf16 partials per batch (fp32 accumulate) and adds bo.

Design notes (vs the transposed-attn.V baseline):
- attn.V in out[q,d] form halves its PE cost (matmul cost ~ moving free
  size: N=64 instead of N=q=512 per stationary), PE ~140us vs ~165us.
- The exp stream is the bottleneck and runs on ACT alone (~134us,
  gapless). Softmax denominators come from DVE tensor_scalar(mult 1.0,
  op1=add, accum_out) on the bf16 P' in SBUF, which hits the 4x DVE
  mode (~330ns per [128,1024] chunk vs 1127ns for tensor_reduce); only
  the final k-tile keeps ACT accum_out (shortest path into the tail).
- GPSIMD must NOT touch PSUM (BIR verifier): every psum->sbuf evacuation
  (qk bias-adds, V copies, attn.V quad drains, transpose copies, proj
  copies) is on DVE/ACT; Pool only does DMA issue, dsum, and V-scaling.
- dma_start occupies the issuing engine ~transfer-time; input DMAs are
  spread over Pool/SP/ACT, outputs ride SP; outputs are bf16.
- PSUM (8 banks): scores 2x[128,1024]f32 + quads/transposes/pair1-V
  2x[128,4,128]f32 + qkv/proj 2x[128,512]f32.
- Schedule: pair0's qkv fillers keep PE ~balanced with ACT; pair1's loop
  carries pair0's tail (last attn.V group, transposes, projection); the
  final tail pipelines quad->transpose->proj->DMA per 4-qt block.
"""

import sys

if "/opt/trn_rl_repo" not in sys.path:
    sys.path.insert(0, "/opt/trn_rl_repo")

import numpy as np
import ml_dtypes

import concourse.bass as bass
import concourse.mybir as mybir
import concourse.tile as tile
from concourse import bacc
from concourse.bass_utils import run_bass_kernel_spmd

F32 = mybir.dt.float32
BF16 = mybir.dt.bfloat16
AF = mybir.ActivationFunctionType

B, S, E, H = 2, 2048, 1024, 16
HL = 4  # heads per core
DH = 64
QK = 512
V3 = 768
NCORES = 8

ET = E // 128  # 8
ST = S // 128  # 16
SC = S // 512  # 4
KT = ST
FG = 4  # k-tiles per attn.V group
NQUAD = 4  # qt's per attn.V psum quad

LAST_RESULTS = None


def build_kernel():
    nc = bacc.Bacc("TRN2", target_bir_lowering=False, debug=False, num_devices=NCORES)

    xT = nc.dram_tensor("xT", [E, S], BF16, kind="ExternalInput")
    wT = nc.dram_tensor("wT", [E, V3], BF16, kind="ExternalInput")
    bq = nc.dram_tensor("bq", [128, 4], F32, kind="ExternalInput")
    bv = nc.dram_tensor("bv", [1, 256], BF16, kind="ExternalInput")
    woT = nc.dram_tensor("woT", [2 * 128, E], BF16, kind="ExternalInput")
    ident = nc.dram_tensor("ident", [128, 128], F32, kind="ExternalInput")
    out0 = nc.dram_tensor("out0", [S, E], BF16, kind="ExternalOutput")
    out1 = nc.dram_tensor("out1", [S, E], BF16, kind="ExternalOutput")
    outd = {0: out0, 1: out1}

    with tile.TileContext(nc) as tc:
        with (
            tc.tile_pool(name="persist", bufs=1) as persist,
            tc.tile_pool(name="smalls", bufs=4) as smalls,
            tc.tile_pool(name="expp", bufs=2 * FG) as expp,
            tc.tile_pool(name="vsp", bufs=2 * FG + 2) as vsp,
            tc.tile_pool(name="fout", bufs=4) as foutp,
            tc.tile_pool(name="mm_ps", bufs=2, space="PSUM") as mm_ps,
            tc.tile_pool(name="sp_ps", bufs=2, space="PSUM") as sp_ps,
            tc.tile_pool(name="ot_ps", bufs=2, space="PSUM") as ot_ps,
        ):
            qk_sb = persist.tile([128, 4, S], BF16, tag="qk")
            v_sb = persist.tile([128, ST, 256], F32, tag="v")
            out_sb = persist.tile([128, 2, ST, 128], F32, tag="out")
            outT_bf = persist.tile([128, 2, S], BF16, tag="outT")
            bq_sb = persist.tile([128, 4], F32, tag="bq")
            bv_sb = persist.tile([1, 256], BF16, tag="bv")
            ones_sb = persist.tile([1, 128], BF16, tag="ones")
            id_sb = persist.tile([128, 128], F32, tag="ident")
            xt_sb = persist.tile([128, ET, S], BF16, tag="xt")
            wt_sb = persist.tile([128, ET, V3], BF16, tag="wt")
            wo_sb = persist.tile([128, 2, E], BF16, tag="wo")

            nc.vector.memset(ones_sb[:], 1.0)

            def dma_xt(sc, et):
                nc.gpsimd.dma_start(
                    xt_sb[:, et, sc * 512 : (sc + 1) * 512],
                    xT[et * 128 : (et + 1) * 128, sc * 512 : (sc + 1) * 512],
                )

            # dma_start occupies the ISSUING engine for ~the transfer time,
            # so spread input DMAs across gpsimd/SP/ACT: the critical
            # wt/sc0/sc1 chunks land in parallel within ~5us. ACT's batch
            # (sc1) sits before its exp stream and finishes by ~4.5us.
            nc.sync.dma_start(bq_sb[:], bq[:])
            for et in range(ET):
                nc.gpsimd.dma_start(wt_sb[:, et, :], wT[et * 128 : (et + 1) * 128, :])
            for et in range(ET):
                nc.sync.dma_start(
                    xt_sb[:, et, 0:512], xT[et * 128 : (et + 1) * 128, 0:512]
                )
            for et in range(ET):
                nc.scalar.dma_start(
                    xt_sb[:, et, 512:1024], xT[et * 128 : (et + 1) * 128, 512:1024]
                )
            for et in range(5):
                nc.scalar.dma_start(
                    xt_sb[:, et, 1024:1536], xT[et * 128 : (et + 1) * 128, 1024:1536]
                )
            for et in range(5):
                nc.sync.dma_start(
                    xt_sb[:, et, 1536:2048], xT[et * 128 : (et + 1) * 128, 1536:2048]
                )
            for et in range(5, ET):
                nc.gpsimd.dma_start(
                    xt_sb[:, et, 1024:1536], xT[et * 128 : (et + 1) * 128, 1024:1536]
                )
                nc.gpsimd.dma_start(
                    xt_sb[:, et, 1536:2048], xT[et * 128 : (et + 1) * 128, 1536:2048]
                )
            nc.sync.dma_start(bv_sb[:], bv[:])
            nc.sync.dma_start(id_sb[:], ident[:])
            for p in range(2):
                nc.sync.dma_start(wo_sb[:, p, :], woT[p * 128 : (p + 1) * 128, :])

            # ---- qkv projection group emitters -------------------------
            # (GPSIMD cannot touch PSUM on HW: bias-adds/copies go to DVE)
            def emit_qk_group(eo, sc, bias_dve=True):
                pt = mm_ps.tile([128, 512], F32, tag="mmps")
                for et in range(ET):
                    nc.tensor.matmul(
                        pt[:],
                        wt_sb[:, et, eo * 128 : (eo + 1) * 128],
                        xt_sb[:, et, sc * 512 : (sc + 1) * 512],
                        start=(et == 0),
                        stop=(et == ET - 1),
                    )
                nc.vector.tensor_scalar_add(
                    qk_sb[:, eo, sc * 512 : (sc + 1) * 512],
                    in0=pt[:],
                    scalar1=bq_sb[:, eo : eo + 1],
                )

            def emit_v_group(st, p):
                # half V-projection: this pair's 128 v-dims only. pair1's
                # groups borrow the otps psum tag (mm is busy with pair0 proj)
                v0 = QK + p * 128
                if p == 1:
                    ptt = ot_ps.tile([128, NQUAD, 128], F32, tag="otps")
                    pt = ptt[:, 0, :]
                else:
                    ptt = mm_ps.tile([128, 512], F32, tag="mmps")
                    pt = ptt[:, 0:128]
                for et in range(ET):
                    nc.tensor.matmul(
                        pt,
                        xt_sb[:, et, st * 128 : (st + 1) * 128],
                        wt_sb[:, et, v0 : v0 + 128],
                        start=(et == 0),
                        stop=False,
                    )
                nc.tensor.matmul(  # + ones^T bv (bias row)
                    pt,
                    ones_sb[0:1, :],
                    bv_sb[0:1, p * 128 : (p + 1) * 128],
                    start=False,
                    stop=True,
                )
                nc.vector.tensor_copy(v_sb[:, st, p * 128 : (p + 1) * 128], pt)

            def qg(eo, sc):
                return lambda: emit_qk_group(eo, sc)

            def vg(st, p=0):
                return lambda: emit_v_group(st, p)

            # ---- attn.V quad (4 qt's of one group) ---------------------
            GROUPS = [(0, 4), (4, 8), (8, 12), (12, 16)]

            def emit_quad(p, g, o, exs, vss):
                k0, k1 = GROUPS[g]
                ot = ot_ps.tile([128, NQUAD, 128], F32, tag="otps")
                for qi in range(NQUAD):
                    qt = NQUAD * o + qi
                    for hh in range(2):
                        for kt in range(k0, k1):
                            nc.tensor.matmul(
                                ot[:, qi, hh * 64 : (hh + 1) * 64],
                                exs[kt][:, hh, qt * 128 : (qt + 1) * 128],
                                vss[kt][:, hh, :],
                                start=(kt == k0),
                                stop=(kt == k1 - 1),
                            )
                dst = out_sb[:, p, NQUAD * o : NQUAD * (o + 1), :]
                if g == 0:
                    nc.vector.tensor_copy(dst, ot[:])
                else:
                    nc.vector.tensor_add(dst, dst, ot[:])

            # ---- transpose block (4 qt's -> outT columns) --------------
            def emit_tblock(p, blk, tail=False):
                # f32 PE transpose straight from out_sb; the psum->sbuf copy
                # does the bf16 conversion (DVE; half on ACT at the tail)
                tp = ot_ps.tile([128, NQUAD, 128], F32, tag="otps")
                for qi in range(NQUAD):
                    nc.tensor.transpose(
                        tp[:, qi, :],
                        out_sb[:, p, NQUAD * blk + qi, :],
                        id_sb[:],
                    )
                for qi in range(NQUAD):
                    qt = NQUAD * blk + qi
                    eng = nc.scalar.copy if (tail and qi % 2 == 0) else (
                        lambda o, i: nc.vector.tensor_copy(o, i)
                    )
                    eng(
                        outT_bf[:, p, qt * 128 : (qt + 1) * 128],
                        tp[:, qi, :],
                    )

            # ---- output projection group (one s-tile, one pair) --------
            def emit_d_group(p, st, tail):
                if not tail:
                    # pair0 proj during pair1's loop: mm psum, copies on DVE
                    # (GPSIMD can't read PSUM), DMA on the idle SP
                    ot = foutp.tile([128, E], BF16, tag="fout", name=f"fo_{p}_{st}")
                    for nck in range(2):
                        pt = mm_ps.tile(
                            [128, 512], F32, tag="mmps", name=f"fp_{p}_{st}_{nck}"
                        )
                        nc.tensor.matmul(
                            pt[:],
                            outT_bf[:, p, st * 128 : (st + 1) * 128],
                            wo_sb[:, p, nck * 512 : (nck + 1) * 512],
                            start=True,
                            stop=True,
                        )
                        nc.vector.tensor_copy(
                            ot[:, nck * 512 : (nck + 1) * 512], pt[:]
                        )
                    nc.sync.dma_start(outd[p][st * 128 : (st + 1) * 128, :], ot[:])
                    return
                # tail (out1, bf16): psum alternates the freed scores slots
                # and the mm slots; both copies of a group go to ACT or DVE
                ot = foutp.tile([128, E], BF16, tag="fout", name=f"fo_{p}_{st}")
                if st % 3 == 2:
                    pts = [
                        mm_ps.tile([128, 512], F32, tag="mmps", name=f"fp_{st}_{n}")
                        for n in range(2)
                    ]
                    chunks = [pts[0][:], pts[1][:]]
                else:
                    pt = sp_ps.tile([128, 1024], F32, tag="sp", name=f"fp_{st}")
                    chunks = [pt[:, 0:512], pt[:, 512:1024]]
                for nck in range(2):
                    nc.tensor.matmul(
                        chunks[nck],
                        outT_bf[:, p, st * 128 : (st + 1) * 128],
                        wo_sb[:, p, nck * 512 : (nck + 1) * 512],
                        start=True,
                        stop=True,
                    )
                if st % 3 != 2 and st != KT - 1:
                    # contiguous psum: single wide copy; ACT takes ~7 of 11
                    # (DVE also carries the drains and transpose copies)
                    if st in (1, 6, 10, 13):
                        nc.vector.tensor_copy(ot[:], pt[:])
                    else:
                        nc.scalar.copy(ot[:], pt[:])
                else:
                    # mm-pair groups + the final group: split across engines
                    for nck in range(2):
                        eng = nc.scalar.copy if nck == 0 else (
                            lambda o, i: nc.vector.tensor_copy(o, i)
                        )
                        eng(ot[:, nck * 512 : (nck + 1) * 512], chunks[nck])
                nc.sync.dma_start(outd[p][st * 128 : (st + 1) * 128, :], ot[:])

            # ---- pre-attention: just enough for pair0 kt0 --------------
            # only what the FIRST 512-wide exp needs; Q-sc1 is emitted
            # between the two split chunks inside kt0
            emit_qk_group(0, 0)  # Q pair0, q cols 0-511
            emit_qk_group(2, 0)  # K pair0, k tiles 0-3


            # pair0 fillers (popped per-kt by pattern below). V-projection is
            # split per pair: pair0's halves here, pair1's in its own loop.
            fillers0 = [
                vg(0), vg(1), vg(2), qg(2, 1), vg(3), vg(4), vg(5), qg(2, 2),
                vg(6), vg(7), vg(8), qg(2, 3), vg(9), vg(10), vg(11), qg(3, 0),
                vg(12), vg(13), vg(14), vg(15),
                qg(1, 0), qg(1, 1), qg(1, 2), qg(1, 3),
            ]
            fillers0.reverse()
            pops0 = [1, 3, 3, 2, 2, 2, 2, 2, 1, 1, 1, 1, 1, 1, 1, 0]  # = 24

            # pair1 in-loop fillers: its V halves + leftover K groups
            fillers1 = [
                vg(0, 1), vg(1, 1), qg(3, 1), vg(2, 1), vg(3, 1), vg(4, 1),
                qg(3, 2), vg(5, 1), vg(6, 1), vg(7, 1), qg(3, 3), vg(8, 1),
                vg(9, 1), vg(10, 1), vg(11, 1), vg(12, 1), vg(13, 1),
                vg(14, 1), vg(15, 1),
            ]
            fillers1.reverse()
            pops1 = [2, 2, 2, 2, 1, 2, 1, 2, 1, 1, 1, 1, 1, 0, 0, 0]  # = 19
            # pair0 proj schedule over pair1 kts 3..15
            proj0 = [2, 1, 1, 1, 1, 1, 1, 1, 1, 2, 1, 2, 1]  # sums to 16

            exs_p0 = vss_p0 = None
            for p in range(2):
                exs = {}
                vss = {}
                proj_done = 0
                def emit_chunk(kt, ex, den, half, hh):
                    sp = sp_ps.tile([128, 1024], F32, tag="sp", name="sp")
                    for qc in range(2):
                        q0 = half * 1024 + qc * 512
                        nc.tensor.matmul(
                            sp[:, qc * 512 : (qc + 1) * 512],
                            qk_sb[
                                hh * 64 : (hh + 1) * 64, 2 + p, kt * 128 : (kt + 1) * 128
                            ],
                            qk_sb[hh * 64 : (hh + 1) * 64, p, q0 : q0 + 512],
                            start=True,
                            stop=True,
                        )
                    exc = ex[:, hh, half * 1024 : (half + 1) * 1024]
                    nc.scalar.activation(exc, sp[:], AF.Exp, scale=0.125)
                    nc.vector.tensor_scalar(
                        out=exc,
                        in0=exc,
                        scalar1=1.0,
                        scalar2=None,
                        op0=mybir.AluOpType.mult,
                        op1=mybir.AluOpType.add,
                        accum_out=den[:, hh, half : half + 1],
                    )

                den2_kt0 = None
                if p == 0:
                    # hand-rolled kts 0-1: interleave both k-tiles' half0
                    # chunks with the remaining Q-group emissions so ACT
                    # never runs dry while PE grinds through qk01/02/03
                    ex0 = expp.tile([128, 2, S], BF16, tag="exp", name="ex0")
                    exs[0] = ex0
                    den0 = smalls.tile([128, 2, 2], F32, tag="den")
                    den2_kt0 = smalls.tile([128, 1], F32, tag="den2")
                    sp = sp_ps.tile([128, 1024], F32, tag="sp", name="sp00")
                    for qc in range(2):
                        nc.tensor.matmul(
                            sp[:, qc * 512 : (qc + 1) * 512],
                            qk_sb[0:64, 2, 0:128],
                            qk_sb[0:64, 0, qc * 512 : (qc + 1) * 512],
                            start=True,
                            stop=True,
                        )
                        dst = den0[:, 0, 0:1] if qc == 0 else den2_kt0[:]
                        nc.scalar.activation(
                            ex0[:, 0, qc * 512 : (qc + 1) * 512],
                            sp[:, qc * 512 : (qc + 1) * 512],
                            AF.Exp,
                            scale=0.125,
                            accum_out=dst,
                        )
                        if qc == 0:
                            emit_qk_group(0, 1)  # Q cols 512-1023
                    emit_chunk(0, ex0, den0, 0, 1)
                    ex1 = expp.tile([128, 2, S], BF16, tag="exp", name="ex1")
                    exs[1] = ex1
                    den1 = smalls.tile([128, 2, 2], F32, tag="den")
                    emit_chunk(1, ex1, den1, 0, 0)
                    emit_qk_group(0, 2)  # Q cols 1024-1535
                    emit_chunk(1, ex1, den1, 0, 1)
                    emit_qk_group(0, 3)  # Q cols 1536-2047
                    for kt_, ex_, den_ in ((0, ex0, den0), (1, ex1, den1)):
                        for hh in range(2):
                            emit_chunk(kt_, ex_, den_, 1, hh)
                    for _ in range(pops0[0] + pops0[1]):
                        if fillers0:
                            fillers0.pop()()
                    for kt_, den_ in ((0, den0), (1, den1)):
                        dsum = smalls.tile([128, 2], F32, tag="dsum")
                        nc.gpsimd.tensor_add(
                            dsum[:], den_[:, :, 0], den_[:, :, 1]
                        )
                        if kt_ == 0:
                            nc.gpsimd.tensor_add(
                                dsum[:, 0:1], dsum[:, 0:1], den2_kt0[:]
                            )
                        rec = smalls.tile([128, 2], F32, tag="rec")
                        nc.vector.reciprocal(rec[:], dsum[:])
                        vs = vsp.tile([128, 2, DH], BF16, tag="vs")
                        vss[kt_] = vs
                        for hh in range(2):
                            nc.gpsimd.tensor_scalar_mul(
                                vs[:, hh, :],
                                in0=v_sb[
                                    :, kt_, (2 * p + hh) * 64 : (2 * p + hh + 1) * 64
                                ],
                                scalar1=rec[:, hh : hh + 1],
                            )

                for kt in range(2 if p == 0 else 0, KT):
                    ex = expp.tile([128, 2, S], BF16, tag="exp")
                    exs[kt] = ex
                    den = smalls.tile([128, 2, 2], F32, tag="den")
                    den2 = None
                    for half in range(2):
                        for hh in range(2):
                            sp = sp_ps.tile([128, 1024], F32, tag="sp")
                            for qc in range(2):
                                q0 = half * 1024 + qc * 512
                                nc.tensor.matmul(
                                    sp[:, qc * 512 : (qc + 1) * 512],
                                    qk_sb[
                                        hh * 64 : (hh + 1) * 64,
                                        2 + p,
                                        kt * 128 : (kt + 1) * 128,
                                    ],
                                    qk_sb[hh * 64 : (hh + 1) * 64, p, q0 : q0 + 512],
                                    start=True,
                                    stop=True,
                                )
                            exc = ex[:, hh, half * 1024 : (half + 1) * 1024]
                            dslice = den[:, hh, half : half + 1]
                            # offload softmax-denominator accumulation to DVE
                            # for ~40 of each pair's 64 chunks (ACT/DVE
                            # balance); keep the final k-tile on ACT so the
                            # tail's first quad isn't gated on a DVE reduce
                            # denominators via DVE tensor_scalar(mult 1.0)
                            # with fused accum: 4x DVE mode makes this ~330ns
                            # so ACT keeps only the exps. Final k-tile stays
                            # on ACT accum (shortest path into the tail).
                            offload = True
                            if offload:
                                nc.scalar.activation(exc, sp[:], AF.Exp, scale=0.125)
                                nc.vector.tensor_scalar(
                                    out=exc,
                                    in0=exc,
                                    scalar1=1.0,
                                    scalar2=None,
                                    op0=mybir.AluOpType.mult,
                                    op1=mybir.AluOpType.add,
                                    accum_out=dslice,
                                )
                            else:
                                nc.scalar.activation(
                                    exc, sp[:], AF.Exp, scale=0.125, accum_out=dslice
                                )
                        if kt == 0 and half == 0 and hh == 1 and p == 0:
                            # Q cols 1024-2047 for pair0 (needed by half1);
                            # pair1's Q groups all complete during pair0.
                            emit_qk_group(0, 2)
                            emit_qk_group(0, 3)

                    # attn.V quad of the previous group (or pair0 spill)
                    if kt >= FG:
                        g, o = kt // FG - 1, kt % FG
                        emit_quad(p, g, o, exs, vss)
                    elif p == 1:
                        emit_quad(0, 3, kt, exs_p0, vss_p0)

                    # fillers / pair0-tail / proj interleaves
                    if p == 0:
                        for _ in range(pops0[kt]):
                            if fillers0:
                                fillers0.pop()()
                    else:
                        for _ in range(pops1[kt]):
                            if fillers1:
                                fillers1.pop()()
                        if 2 <= kt <= 5:
                            emit_tblock(0, kt - 2)
                        if kt >= 3:
                            for _ in range(proj0[kt - 3]):
                                if proj_done < NQUAD * (kt - 1):
                                    emit_d_group(0, proj_done, tail=False)
                                    proj_done += 1

                    # denominators -> 1/denom -> scaled V for this k-tile
                    # (combine/scale on Pool; reciprocal is DVE-only)
                    dsum = smalls.tile([128, 2], F32, tag="dsum")
                    nc.gpsimd.tensor_add(dsum[:], den[:, :, 0], den[:, :, 1])
                    if den2 is not None:
                        nc.gpsimd.tensor_add(dsum[:, 0:1], dsum[:, 0:1], den2[:])
                    rec = smalls.tile([128, 2], F32, tag="rec")
                    nc.vector.reciprocal(rec[:], dsum[:])
                    vs = vsp.tile([128, 2, DH], BF16, tag="vs")
                    vss[kt] = vs
                    for hh in range(2):
                        nc.gpsimd.tensor_scalar_mul(
                            vs[:, hh, :],
                            in0=v_sb[:, kt, (2 * p + hh) * 64 : (2 * p + hh + 1) * 64],
                            scalar1=rec[:, hh : hh + 1],
                        )

                if p == 0:
                    exs_p0, vss_p0 = exs, vss
                else:
                    # tail: stay one quad ahead so PE never idles while a
                    # block's drain/convert hop across DVE/ACT
                    emit_quad(1, 3, 0, exs, vss)
                    for o in range(NQUAD):
                        if o + 1 < NQUAD:
                            emit_quad(1, 3, o + 1, exs, vss)
                        emit_tblock(1, o, tail=True)
                        for st in range(NQUAD * o, NQUAD * (o + 1)):
                            emit_d_group(1, st, tail=True)

    nc.compile()
    return nc


def _shard_inputs(input, Wqkv, bqkv, Wo):
    """Build the 8 per-core input dicts (host-side layout/sharding)."""
    bf16 = ml_dtypes.bfloat16
    ident_f32 = np.eye(128, dtype=np.float32)
    in_maps = []
    for c in range(NCORES):
        b = c // 4
        g = c % 4
        heads = range(4 * g, 4 * g + 4)
        rows = (
            [slice(64 * h, 64 * h + 64) for h in heads]
            + [slice(E + 64 * h, E + 64 * h + 64) for h in heads]
            + [slice(2 * E + 64 * h, 2 * E + 64 * h + 64) for h in heads]
        )
        W_sel = np.concatenate([Wqkv[s] for s in rows], axis=0)  # [768, 1024]
        b_sel = np.concatenate([bqkv[s] for s in rows], axis=0)  # [768]
        in_maps.append(
            {
                "xT": np.ascontiguousarray(input[b].T).astype(bf16),
                "wT": np.ascontiguousarray(W_sel.T).astype(bf16),
                "bq": np.ascontiguousarray(b_sel[:QK].reshape(4, 128).T),
                "bv": np.ascontiguousarray(b_sel[QK:V3].reshape(1, 256)).astype(bf16),
                "woT": np.ascontiguousarray(
                    Wo[:, 4 * g * DH : 4 * (g + 1) * DH].T
                ).astype(bf16),
                "ident": ident_f32,
            }
        )
    return in_maps


def kernel(input, Wqkv, bqkv, Wo, bo, _trace=False):
    global LAST_RESULTS
    input = np.asarray(input, dtype=np.float32)
    Wqkv = np.asarray(Wqkv, dtype=np.float32)
    bqkv = np.asarray(bqkv, dtype=np.float32)
    Wo = np.asarray(Wo, dtype=np.float32)
    bo = np.asarray(bo, dtype=np.float32)

    nc = build_kernel()
    in_maps = _shard_inputs(input, Wqkv, bqkv, Wo)
    kwargs = {}
    if _trace:
        kwargs = dict(trace=True, trace_cores=[0])
    try:
        res = run_bass_kernel_spmd(nc, in_maps, core_ids=list(range(NCORES)), **kwargs)
    except ModuleNotFoundError:
        # no NTFF profiling hook in this container — run without trace
        res = run_bass_kernel_spmd(nc, in_maps, core_ids=list(range(NCORES)))
    LAST_RESULTS = res

    out = np.zeros((B, S, E), dtype=np.float32)
    for c in range(NCORES):
        out[c // 4] += res.results[c]["out0"]
        out[c // 4] += res.results[c]["out1"]
    out += bo
    return out


# revision 14
# speedup vs baseline: 1.0164x; 1.0038x over previous
"""Multi-head attention (softmax over the QUERY axis) on 8 TRN2 NeuronCores.

Sharding: 2 batches x 4 head-groups (4 heads each) -> 8 cores; each core
processes its 4 heads as two head PAIRS (p=0,1).

Per (batch b, head pair p) on a core:
    qkT = W_{q,k} @ x_b^T + b_{q,k}        [512, 2048]  (e_out on partitions)
    V   = x_b @ W_v^T + b_v                [2048, 256]  (per-pair halves)
    S'  = K Q^T  (scores TRANSPOSED)       [k, q] per head
    P'  = exp(S'/8), denom[k] = sum_q P'   (ACT accum_out or DVE tensor_reduce)
    V'  = V[k,:]/denom[k]                  (scaled per k-tile)
    out[q, d] = sum_k P'[k,q] V'[k,d]      <-- P' stationary, V' moving (N=64)
    outT = f32 PE identity-transpose of out, bf16 on psum->sbuf copy
    part = outT^T @ WoT_p                  [2048, 1024] partial per pair, bf16
Host sums the 8 bf16 partials per batch (fp32 accumulate) and adds bo.

Design notes (vs the transposed-attn.V baseline):
- attn.V in out[q,d] form halves its PE cost (matmul cost ~ moving free
  size: N=64 instead of N=q=512 per stationary), PE ~140us vs ~165us.
- The exp stream is the bottleneck and runs on ACT alone (~134us,
  gapless). Softmax denominators come from DVE tensor_scalar(mult 1.0,
  op1=add, accum_out) on the bf16 P' in SBUF, which hits the 4x DVE
  mode (~330ns per [128,1024] chunk vs 1127ns for tensor_reduce); only
  the final k-tile keeps ACT accum_out (shortest path into the tail).
- GPSIMD must NOT touch PSUM (BIR verifier): every psum->sbuf evacuation
  (qk bias-adds, V copies, attn.V quad drains, transpose copies, proj
  copies) is on DVE/ACT; Pool only does DMA issue, dsum, and V-scaling.
- dma_start occupies the issuing engine ~transfer-time; input DMAs are
  spread over Pool/SP/ACT, outputs ride SP; outputs are bf16.
- PSUM (8 banks): scores 2x[128,1024]f32 + quads/transposes/pair1-V
  2x[128,4,128]f32 + qkv/proj 2x[128,512]f32.
- Schedule: pair0's qkv fillers keep PE ~balanced with ACT; pair1's loop
  carries pair0's tail (last attn.V group, transposes, projection); the
  final tail pipelines quad->transpose->proj->DMA per 4-qt block.
"""

import sys

if "/opt/trn_rl_repo" not in sys.path:
    sys.path.insert(0, "/opt/trn_rl_repo")

import numpy as np
import ml_dtypes

import concourse.bass as bass
import concourse.mybir as mybir
import concourse.tile as tile
from concourse import bacc
from concourse.bass_utils import run_bass_kernel_spmd

F32 = mybir.dt.float32
BF16 = mybir.dt.bfloat16
AF = mybir.ActivationFunctionType

B, S, E, H = 2, 2048, 1024, 16
HL = 4  # heads per core
DH = 64
QK = 512
V3 = 768
NCORES = 8

ET = E // 128  # 8
ST = S // 128  # 16
SC = S // 512  # 4
KT = ST
FG = 4  # k-tiles per attn.V group
NQUAD = 4  # qt's per attn.V psum quad

LAST_RESULTS = None


def build_kernel():
    nc = bacc.Bacc("TRN2", target_bir_lowering=False, debug=False, num_devices=NCORES)

    xT = nc.dram_tensor("xT", [E, S], BF16, kind="ExternalInput")
    wT = nc.dram_tensor("wT", [E, V3], BF16, kind="ExternalInput")
    bq = nc.dram_tensor("bq", [128, 4], F32, kind="ExternalInput")
    bv = nc.dram_tensor("bv", [1, 256], BF16, kind="ExternalInput")
    woT = nc.dram_tensor("woT", [2 * 128, E], BF16, kind="ExternalInput")
    ident = nc.dram_tensor("ident", [128, 128], F32, kind="ExternalInput")
    out0 = nc.dram_tensor("out0", [S, E], BF16, kind="ExternalOutput")
    out1 = nc.dram_tensor("out1", [S, E], BF16, kind="ExternalOutput")
    outd = {0: out0, 1: out1}

    with tile.TileContext(nc) as tc:
        with (
            tc.tile_pool(name="persist", bufs=1) as persist,
            tc.tile_pool(name="smalls", bufs=4) as smalls,
            tc.tile_pool(name="expp", bufs=2 * FG) as expp,
            tc.tile_pool(name="vsp", bufs=2 * FG + 2) as vsp,
            tc.tile_pool(name="fout", bufs=4) as foutp,
            tc.tile_pool(name="mm_ps", bufs=2, space="PSUM") as mm_ps,
            tc.tile_pool(name="sp_ps", bufs=2, space="PSUM") as sp_ps,
            tc.tile_pool(name="ot_ps", bufs=2, space="PSUM") as ot_ps,
        ):
            qk_sb = persist.tile([128, 4, S], BF16, tag="qk")
            v_sb = persist.tile([128, ST, 256], F32, tag="v")
            out_sb = persist.tile([128, 2, ST, 128], F32, tag="out")
            outT_bf = persist.tile([128, 2, S], BF16, tag="outT")
            bq_sb = persist.tile([128, 4], F32, tag="bq")
            bv_sb = persist.tile([1, 256], BF16, tag="bv")
            ones_sb = persist.tile([1, 128], BF16, tag="ones")
            id_sb = persist.tile([128, 128], F32, tag="ident")
            xt_sb = persist.tile([128, ET, S], BF16, tag="xt")
            wt_sb = persist.tile([128, ET, V3], BF16, tag="wt")
            wo_sb = persist.tile([128, 2, E], BF16, tag="wo")

            nc.vector.memset(ones_sb[:], 1.0)

            def dma_xt(sc, et):
                nc.gpsimd.dma_start(
                    xt_sb[:, et, sc * 512 : (sc + 1) * 512],
                    xT[et * 128 : (et + 1) * 128, sc * 512 : (sc + 1) * 512],
                )

            # dma_start occupies the ISSUING engine for ~the transfer time,
            # so spread input DMAs across gpsimd/SP/ACT: the critical
            # wt/sc0/sc1 chunks land in parallel within ~5us. ACT's batch
            # (sc1) sits before its exp stream and finishes by ~4.5us.
            nc.sync.dma_start(bq_sb[:], bq[:])
            for et in range(ET):
                nc.gpsimd.dma_start(wt_sb[:, et, :], wT[et * 128 : (et + 1) * 128, :])
            for et in range(ET):
                nc.sync.dma_start(
                    xt_sb[:, et, 0:512], xT[et * 128 : (et + 1) * 128, 0:512]
                )
            for et in range(ET):
                nc.scalar.dma_start(
                    xt_sb[:, et, 512:1024], xT[et * 128 : (et + 1) * 128, 512:1024]
                )
            for et in range(5):
                nc.scalar.dma_start(
                    xt_sb[:, et, 1024:1536], xT[et * 128 : (et + 1) * 128, 1024:1536]
                )
            for et in range(5):
                nc.sync.dma_start(
                    xt_sb[:, et, 1536:2048], xT[et * 128 : (et + 1) * 128, 1536:2048]
                )
            for et in range(5, ET):
                nc.gpsimd.dma_start(
                    xt_sb[:, et, 1024:1536], xT[et * 128 : (et + 1) * 128, 1024:1536]
                )
                nc.gpsimd.dma_start(
                    xt_sb[:, et, 1536:2048], xT[et * 128 : (et + 1) * 128, 1536:2048]
                )
            nc.sync.dma_start(bv_sb[:], bv[:])
            nc.sync.dma_start(id_sb[:], ident[:])
            for p in range(2):
                nc.sync.dma_start(wo_sb[:, p, :], woT[p * 128 : (p + 1) * 128, :])

            # ---- qkv projection group emitters -------------------------
            # (GPSIMD cannot touch PSUM on HW: bias-adds/copies go to DVE)
            def emit_qk_group(eo, sc, bias_dve=True):
                pt = mm_ps.tile([128, 512], F32, tag="mmps")
                for et in range(ET):
                    nc.tensor.matmul(
                        pt[:],
                        wt_sb[:, et, eo * 128 : (eo + 1) * 128],
                        xt_sb[:, et, sc * 512 : (sc + 1) * 512],
                        start=(et == 0),
                        stop=(et == ET - 1),
                    )
                nc.vector.tensor_scalar_add(
                    qk_sb[:, eo, sc * 512 : (sc + 1) * 512],
                    in0=pt[:],
                    scalar1=bq_sb[:, eo : eo + 1],
                )

            def emit_v_group(st, p):
                # half V-projection: this pair's 128 v-dims only. pair1's
                # groups borrow the otps psum tag (mm is busy with pair0 proj)
                v0 = QK + p * 128
                if p == 1:
                    ptt = ot_ps.tile([128, NQUAD, 128], F32, tag="otps")
                    pt = ptt[:, 0, :]
                else:
                    ptt = mm_ps.tile([128, 512], F32, tag="mmps")
                    pt = ptt[:, 0:128]
                for et in range(ET):
                    nc.tensor.matmul(
                        pt,
                        xt_sb[:, et, st * 128 : (st + 1) * 128],
                        wt_sb[:, et, v0 : v0 + 128],
                        start=(et == 0),
                        stop=False,
                    )
                nc.tensor.matmul(  # + ones^T bv (bias row)
                    pt,
                    ones_sb[0:1, :],
                    bv_sb[0:1, p * 128 : (p + 1) * 128],
                    start=False,
                    stop=True,
                )
                nc.vector.tensor_copy(v_sb[:, st, p * 128 : (p + 1) * 128], pt)

            def qg(eo, sc):
                return lambda: emit_qk_group(eo, sc)

            def vg(st, p=0):
                return lambda: emit_v_group(st, p)

            # ---- attn.V quad (4 qt's of one group) ---------------------
            GROUPS = [(0, 4), (4, 8), (8, 12), (12, 16)]

            def emit_quad(p, g, o, exs, vss, per_qt_drain=False):
                k0, k1 = GROUPS[g]
                ot = ot_ps.tile([128, NQUAD, 128], F32, tag="otps")
                for qi in range(NQUAD):
                    qt = NQUAD * o + qi
                    for hh in range(2):
                        for kt in range(k0, k1):
                            nc.tensor.matmul(
                                ot[:, qi, hh * 64 : (hh + 1) * 64],
                                exs[kt][:, hh, qt * 128 : (qt + 1) * 128],
                                vss[kt][:, hh, :],
                                start=(kt == k0),
                                stop=(kt == k1 - 1),
                            )
                    if per_qt_drain:
                        d = out_sb[:, p, qt : qt + 1, :]
                        nc.vector.tensor_add(d, d, ot[:, qi : qi + 1, :])
                if per_qt_drain:
                    return
                dst = out_sb[:, p, NQUAD * o : NQUAD * (o + 1), :]
                if g == 0:
                    nc.vector.tensor_copy(dst, ot[:])
                else:
                    nc.vector.tensor_add(dst, dst, ot[:])

            # ---- transpose block (4 qt's -> outT columns) --------------
            def emit_tblock(p, blk, tail=False):
                # f32 PE transpose straight from out_sb; the psum->sbuf copy
                # does the bf16 conversion (DVE; half on ACT at the tail)
                tp = ot_ps.tile([128, NQUAD, 128], F32, tag="otps")
                for qi in range(NQUAD):
                    nc.tensor.transpose(
                        tp[:, qi, :],
                        out_sb[:, p, NQUAD * blk + qi, :],
                        id_sb[:],
                    )
                for qi in range(NQUAD):
                    qt = NQUAD * blk + qi
                    eng = nc.scalar.copy if (tail and qi % 2 == 0) else (
                        lambda o, i: nc.vector.tensor_copy(o, i)
                    )
                    eng(
                        outT_bf[:, p, qt * 128 : (qt + 1) * 128],
                        tp[:, qi, :],
                    )

            # ---- output projection group (one s-tile, one pair) --------
            def emit_d_group(p, st, tail):
                if not tail:
                    # pair0 proj during pair1's loop: mm psum, copies on DVE
                    # (GPSIMD can't read PSUM), DMA on the idle SP
                    ot = foutp.tile([128, E], BF16, tag="fout", name=f"fo_{p}_{st}")
                    for nck in range(2):
                        pt = mm_ps.tile(
                            [128, 512], F32, tag="mmps", name=f"fp_{p}_{st}_{nck}"
                        )
                        nc.tensor.matmul(
                            pt[:],
                            outT_bf[:, p, st * 128 : (st + 1) * 128],
                            wo_sb[:, p, nck * 512 : (nck + 1) * 512],
                            start=True,
                            stop=True,
                        )
                        nc.vector.tensor_copy(
                            ot[:, nck * 512 : (nck + 1) * 512], pt[:]
                        )
                    nc.sync.dma_start(outd[p][st * 128 : (st + 1) * 128, :], ot[:])
                    return
                # tail (out1, bf16): psum alternates the freed scores slots
                # and the mm slots; both copies of a group go to ACT or DVE
                ot = foutp.tile([128, E], BF16, tag="fout", name=f"fo_{p}_{st}")
                if st % 3 == 2:
                    pts = [
                        mm_ps.tile([128, 512], F32, tag="mmps", name=f"fp_{st}_{n}")
                        for n in range(2)
                    ]
                    chunks = [pts[0][:], pts[1][:]]
                else:
                    pt = sp_ps.tile([128, 1024], F32, tag="sp", name=f"fp_{st}")
                    chunks = [pt[:, 0:512], pt[:, 512:1024]]
                for nck in range(2):
                    nc.tensor.matmul(
                        chunks[nck],
                        outT_bf[:, p, st * 128 : (st + 1) * 128],
                        wo_sb[:, p, nck * 512 : (nck + 1) * 512],
                        start=True,
                        stop=True,
                    )
                if st % 3 != 2 and st != KT - 1:
                    # contiguous psum: single wide copy; ACT takes ~7 of 11
                    # (DVE also carries the drains and transpose copies)
                    if st in (1, 6, 10, 13):
                        nc.vector.tensor_copy(ot[:], pt[:])
                    else:
                        nc.scalar.copy(ot[:], pt[:])
                else:
                    # mm-pair groups + the final group: split across engines
                    for nck in range(2):
                        eng = nc.scalar.copy if nck == 0 else (
                            lambda o, i: nc.vector.tensor_copy(o, i)
                        )
                        eng(ot[:, nck * 512 : (nck + 1) * 512], chunks[nck])
                nc.sync.dma_start(outd[p][st * 128 : (st + 1) * 128, :], ot[:])

            # ---- pre-attention: just enough for pair0 kt0 --------------
            # only what the FIRST 512-wide exp needs; Q-sc1 is emitted
            # between the two split chunks inside kt0
            emit_qk_group(0, 0)  # Q pair0, q cols 0-511
            emit_qk_group(2, 0)  # K pair0, k tiles 0-3


            # pair0 fillers (popped per-kt by pattern below). V-projection is
            # split per pair: pair0's halves here, pair1's in its own loop.
            fillers0 = [
                vg(0), vg(1), vg(2), qg(2, 1), vg(3), vg(4), vg(5), qg(2, 2),
                vg(6), vg(7), vg(8), qg(2, 3), vg(9), vg(10), vg(11), qg(3, 0),
                vg(12), vg(13), vg(14), vg(15),
                qg(1, 0), qg(1, 1), qg(1, 2), qg(1, 3),
            ]
            fillers0.reverse()
            pops0 = [1, 3, 3, 2, 2, 2, 2, 2, 1, 1, 1, 1, 1, 1, 1, 0]  # = 24

            # pair1 in-loop fillers: its V halves + leftover K groups
            fillers1 = [
                vg(0, 1), vg(1, 1), qg(3, 1), vg(2, 1), vg(3, 1), vg(4, 1),
                qg(3, 2), vg(5, 1), vg(6, 1), vg(7, 1), qg(3, 3), vg(8, 1),
                vg(9, 1), vg(10, 1), vg(11, 1), vg(12, 1), vg(13, 1),
                vg(14, 1), vg(15, 1),
            ]
            fillers1.reverse()
            pops1 = [2, 2, 2, 2, 1, 2, 1, 2, 1, 1, 1, 1, 1, 0, 0, 0]  # = 19
            # pair0 proj schedule over pair1 kts 3..15
            proj0 = [2, 1, 1, 1, 1, 1, 1, 1, 1, 2, 1, 2, 1]  # sums to 16

            exs_p0 = vss_p0 = None
            for p in range(2):
                exs = {}
                vss = {}
                proj_done = 0
                def emit_chunk(kt, ex, den, half, hh):
                    sp = sp_ps.tile([128, 1024], F32, tag="sp", name="sp")
                    for qc in range(2):
                        q0 = half * 1024 + qc * 512
                        nc.tensor.matmul(
                            sp[:, qc * 512 : (qc + 1) * 512],
                            qk_sb[
                                hh * 64 : (hh + 1) * 64, 2 + p, kt * 128 : (kt + 1) * 128
                            ],
                            qk_sb[hh * 64 : (hh + 1) * 64, p, q0 : q0 + 512],
                            start=True,
                            stop=True,
                        )
                    exc = ex[:, hh, half * 1024 : (half + 1) * 1024]
                    nc.scalar.activation(exc, sp[:], AF.Exp, scale=0.125)
                    nc.vector.tensor_scalar(
                        out=exc,
                        in0=exc,
                        scalar1=1.0,
                        scalar2=None,
                        op0=mybir.AluOpType.mult,
                        op1=mybir.AluOpType.add,
                        accum_out=den[:, hh, half : half + 1],
                    )

                den2_kt0 = None
                if p == 0:
                    # hand-rolled kts 0-1: interleave both k-tiles' half0
                    # chunks with the remaining Q-group emissions so ACT
                    # never runs dry while PE grinds through qk01/02/03
                    ex0 = expp.tile([128, 2, S], BF16, tag="exp", name="ex0")
                    exs[0] = ex0
                    den0 = smalls.tile([128, 2, 2], F32, tag="den")
                    den2_kt0 = smalls.tile([128, 1], F32, tag="den2")
                    sp = sp_ps.tile([128, 1024], F32, tag="sp", name="sp00")
                    for qc in range(2):
                        nc.tensor.matmul(
                            sp[:, qc * 512 : (qc + 1) * 512],
                            qk_sb[0:64, 2, 0:128],
                            qk_sb[0:64, 0, qc * 512 : (qc + 1) * 512],
                            start=True,
                            stop=True,
                        )
                        dst = den0[:, 0, 0:1] if qc == 0 else den2_kt0[:]
                        nc.scalar.activation(
                            ex0[:, 0, qc * 512 : (qc + 1) * 512],
                            sp[:, qc * 512 : (qc + 1) * 512],
                            AF.Exp,
                            scale=0.125,
                            accum_out=dst,
                        )
                        if qc == 0:
                            emit_qk_group(0, 1)  # Q cols 512-1023
                    emit_chunk(0, ex0, den0, 0, 1)
                    ex1 = expp.tile([128, 2, S], BF16, tag="exp", name="ex1")
                    exs[1] = ex1
                    den1 = smalls.tile([128, 2, 2], F32, tag="den")
                    emit_chunk(1, ex1, den1, 0, 0)
                    emit_qk_group(0, 2)  # Q cols 1024-1535
                    emit_chunk(1, ex1, den1, 0, 1)
                    emit_qk_group(0, 3)  # Q cols 1536-2047
                    for kt_, ex_, den_ in ((0, ex0, den0), (1, ex1, den1)):
                        for hh in range(2):
                            emit_chunk(kt_, ex_, den_, 1, hh)
                    for _ in range(pops0[0] + pops0[1]):
                        if fillers0:
                            fillers0.pop()()
                    for kt_, den_ in ((0, den0), (1, den1)):
                        dsum = smalls.tile([128, 2], F32, tag="dsum")
                        nc.gpsimd.tensor_add(
                            dsum[:], den_[:, :, 0], den_[:, :, 1]
                        )
                        if kt_ == 0:
                            nc.gpsimd.tensor_add(
                                dsum[:, 0:1], dsum[:, 0:1], den2_kt0[:]
                            )
                        rec = smalls.tile([128, 2], F32, tag="rec")
                        nc.vector.reciprocal(rec[:], dsum[:])
                        vs = vsp.tile([128, 2, DH], BF16, tag="vs")
                        vss[kt_] = vs
                        for hh in range(2):
                            nc.gpsimd.tensor_scalar_mul(
                                vs[:, hh, :],
                                in0=v_sb[
                                    :, kt_, (2 * p + hh) * 64 : (2 * p + hh + 1) * 64
                                ],
                                scalar1=rec[:, hh : hh + 1],
                            )

                for kt in range(2 if p == 0 else 0, KT):
                    ex = expp.tile([128, 2, S], BF16, tag="exp")
                    exs[kt] = ex
                    den = smalls.tile([128, 2, 2], F32, tag="den")
                    den2 = None
                    for half in range(2):
                        for hh in range(2):
                            sp = sp_ps.tile([128, 1024], F32, tag="sp")
                            for qc in range(2):
                                q0 = half * 1024 + qc * 512
                                nc.tensor.matmul(
                                    sp[:, qc * 512 : (qc + 1) * 512],
                                    qk_sb[
                                        hh * 64 : (hh + 1) * 64,
                                        2 + p,
                                        kt * 128 : (kt + 1) * 128,
                                    ],
                                    qk_sb[hh * 64 : (hh + 1) * 64, p, q0 : q0 + 512],
                                    start=True,
                                    stop=True,
                                )
                            exc = ex[:, hh, half * 1024 : (half + 1) * 1024]
                            dslice = den[:, hh, half : half + 1]
                            # offload softmax-denominator accumulation to DVE
                            # for ~40 of each pair's 64 chunks (ACT/DVE
                            # balance); keep the final k-tile on ACT so the
                            # tail's first quad isn't gated on a DVE reduce
                            # denominators via DVE tensor_scalar(mult 1.0)
                            # with fused accum: 4x DVE mode makes this ~330ns
                            # so ACT keeps only the exps. Final k-tile stays
                            # on ACT accum (shortest path into the tail).
                            offload = True
                            if offload:
                                nc.scalar.activation(exc, sp[:], AF.Exp, scale=0.125)
                                nc.vector.tensor_scalar(
                                    out=exc,
                                    in0=exc,
                                    scalar1=1.0,
                                    scalar2=None,
                                    op0=mybir.AluOpType.mult,
                                    op1=mybir.AluOpType.add,
                                    accum_out=dslice,
                                )
                            else:
                                nc.scalar.activation(
                                    exc, sp[:], AF.Exp, scale=0.125, accum_out=dslice
                                )
                        if kt == 0 and half == 0 and hh == 1 and p == 0:
                            # Q cols 1024-2047 for pair0 (needed by half1);
                            # pair1's Q groups all complete during pair0.
                            emit_qk_group(0, 2)
                            emit_qk_group(0, 3)

                    # attn.V quad of the previous group (or pair0 spill)
                    if kt >= FG:
                        g, o = kt // FG - 1, kt % FG
                        emit_quad(p, g, o, exs, vss)
                    elif p == 1:
                        emit_quad(0, 3, kt, exs_p0, vss_p0)

                    # fillers / pair0-tail / proj interleaves
                    if p == 0:
                        for _ in range(pops0[kt]):
                            if fillers0:
                                fillers0.pop()()
                    else:
                        for _ in range(pops1[kt]):
                            if fillers1:
                                fillers1.pop()()
                        if 2 <= kt <= 5:
                            emit_tblock(0, kt - 2)
                        if kt >= 3:
                            for _ in range(proj0[kt - 3]):
                                if proj_done < NQUAD * (kt - 1):
                                    emit_d_group(0, proj_done, tail=False)
                                    proj_done += 1

                    # denominators -> 1/denom -> scaled V for this k-tile
                    # (combine/scale on Pool; reciprocal is DVE-only)
                    dsum = smalls.tile([128, 2], F32, tag="dsum")
                    nc.gpsimd.tensor_add(dsum[:], den[:, :, 0], den[:, :, 1])
                    if den2 is not None:
                        nc.gpsimd.tensor_add(dsum[:, 0:1], dsum[:, 0:1], den2[:])
                    rec = smalls.tile([128, 2], F32, tag="rec")
                    nc.vector.reciprocal(rec[:], dsum[:])
                    vs = vsp.tile([128, 2, DH], BF16, tag="vs")
                    vss[kt] = vs
                    for hh in range(2):
                        nc.gpsimd.tensor_scalar_mul(
                            vs[:, hh, :],
                            in0=v_sb[:, kt, (2 * p + hh) * 64 : (2 * p + hh + 1) * 64],
                            scalar1=rec[:, hh : hh + 1],
                        )

                if p == 0:
                    exs_p0, vss_p0 = exs, vss
                else:
                    # tail: stay one quad ahead so PE never idles while a
                    # block's drain/convert hop across DVE/ACT
                    emit_quad(1, 3, 0, exs, vss)
                    for o in range(NQUAD):
                        emit_tblock(1, o, tail=True)
                        if o + 1 < NQUAD:
                            emit_quad(1, 3, o + 1, exs, vss)
                        for st in range(NQUAD * o, NQUAD * (o + 1)):
                            emit_d_group(1, st, tail=True)

    nc.compile()
    return nc


def _shard_inputs(input, Wqkv, bqkv, Wo):
    """Build the 8 per-core input dicts (host-side layout/sharding)."""
    bf16 = ml_dtypes.bfloat16
    ident_f32 = np.eye(128, dtype=np.float32)
    in_maps = []
    for c in range(NCORES):
        b = c // 4
        g = c % 4
        heads = range(4 * g, 4 * g + 4)
        rows = (
            [slice(64 * h, 64 * h + 64) for h in heads]
            + [slice(E + 64 * h, E + 64 * h + 64) for h in heads]
            + [slice(2 * E + 64 * h, 2 * E + 64 * h + 64) for h in heads]
        )
        W_sel = np.concatenate([Wqkv[s] for s in rows], axis=0)  # [768, 1024]
        b_sel = np.concatenate([bqkv[s] for s in rows], axis=0)  # [768]
        in_maps.append(
            {
                "xT": np.ascontiguousarray(input[b].T).astype(bf16),
                "wT": np.ascontiguousarray(W_sel.T).astype(bf16),
                "bq": np.ascontiguousarray(b_sel[:QK].reshape(4, 128).T),
                "bv": np.ascontiguousarray(b_sel[QK:V3].reshape(1, 256)).astype(bf16),
                "woT": np.ascontiguousarray(
                    Wo[:, 4 * g * DH : 4 * (g + 1) * DH].T
                ).astype(bf16),
                "ident": ident_f32,
            }
        )
    return in_maps


def kernel(input, Wqkv, bqkv, Wo, bo, _trace=False):
    global LAST_RESULTS
    input = np.asarray(input, dtype=np.float32)
    Wqkv = np.asarray(Wqkv, dtype=np.float32)
    bqkv = np.asarray(bqkv, dtype=np.float32)
    Wo = np.asarray(Wo, dtype=np.float32)
    bo = np.asarray(bo, dtype=np.float32)

    nc = build_kernel()
    in_maps = _shard_inputs(input, Wqkv, bqkv, Wo)
    kwargs = {}
    if _trace:
        kwargs = dict(trace=True, trace_cores=[0])
    try:
        res = run_bass_kernel_spmd(nc, in_maps, core_ids=list(range(NCORES)), **kwargs)
    except ModuleNotFoundError:
        # no NTFF profiling hook in this container — run without trace
        res = run_bass_kernel_spmd(nc, in_maps, core_ids=list(range(NCORES)))
    LAST_RESULTS = res

    out = np.zeros((B, S, E), dtype=np.float32)
    for c in range(NCORES):
        out[c // 4] += res.results[c]["out0"]
        out[c // 4] += res.results[c]["out1"]
    out += bo
    return out


# revision 15
# speedup vs baseline: 1.0207x; 1.0042x over previous
"""Multi-head attention (softmax over the QUERY axis) on 8 TRN2 NeuronCores.

Sharding: 2 batches x 4 head-groups (4 heads each) -> 8 cores; each core
processes its 4 heads as two head PAIRS (p=0,1).

Per (batch b, head pair p) on a core:
    qkT = W_{q,k} @ x_b^T + b_{q,k}        [512, 2048]  (e_out on partitions)
    V   = x_b @ W_v^T + b_v                [2048, 256]  (per-pair halves)
    S'  = K Q^T  (scores TRANSPOSED)       [k, q] per head
    P'  = exp(S'/8), denom[k] = sum_q P'   (ACT accum_out or DVE tensor_reduce)
    V'  = V[k,:]/denom[k]                  (scaled per k-tile)
    out[q, d] = sum_k P'[k,q] V'[k,d]      <-- P' stationary, V' moving (N=64)
    outT = f32 PE identity-transpose of out, bf16 on psum->sbuf copy
    part = outT^T @ WoT_p                  [2048, 1024] partial per pair, bf16
Host sums the 8 bf16 partials per batch (fp32 accumulate) and adds bo.

Design notes (vs the transposed-attn.V baseline):
- attn.V in out[q,d] form halves its PE cost (matmul cost ~ moving free
  size: N=64 instead of N=q=512 per stationary), PE ~140us vs ~165us.
- The exp stream is the bottleneck and runs on ACT alone (~134us,
  gapless). Softmax denominators come from DVE tensor_scalar(mult 1.0,
  op1=add, accum_out) on the bf16 P' in SBUF, which hits the 4x DVE
  mode (~330ns per [128,1024] chunk vs 1127ns for tensor_reduce); only
  the final k-tile keeps ACT accum_out (shortest path into the tail).
- GPSIMD must NOT touch PSUM (BIR verifier): every psum->sbuf evacuation
  (qk bias-adds, V copies, attn.V quad drains, transpose copies, proj
  copies) is on DVE/ACT; Pool only does DMA issue, dsum, and V-scaling.
- dma_start occupies the issuing engine ~transfer-time; input DMAs are
  spread over Pool/SP/ACT, outputs ride SP; outputs are bf16.
- PSUM (8 banks): scores 2x[128,1024]f32 + quads/transposes/pair1-V
  2x[128,4,128]f32 + qkv/proj 2x[128,512]f32.
- Schedule: pair0's qkv fillers keep PE ~balanced with ACT; pair1's loop
  carries pair0's tail (last attn.V group, transposes, projection); the
  final tail pipelines quad->transpose->proj->DMA per 4-qt block.
"""

import sys

if "/opt/trn_rl_repo" not in sys.path:
    sys.path.insert(0, "/opt/trn_rl_repo")

import numpy as np
import ml_dtypes

import concourse.bass as bass
import concourse.mybir as mybir
import concourse.tile as tile
from concourse import bacc
from concourse.bass_utils import run_bass_kernel_spmd

F32 = mybir.dt.float32
BF16 = mybir.dt.bfloat16
AF = mybir.ActivationFunctionType

B, S, E, H = 2, 2048, 1024, 16
HL = 4  # heads per core
DH = 64
QK = 512
V3 = 768
NCORES = 8

ET = E // 128  # 8
ST = S // 128  # 16
SC = S // 512  # 4
KT = ST
FG = 4  # k-tiles per attn.V group
NQUAD = 4  # qt's per attn.V psum quad

LAST_RESULTS = None


def build_kernel():
    nc = bacc.Bacc("TRN2", target_bir_lowering=False, debug=False, num_devices=NCORES)

    xT = nc.dram_tensor("xT", [E, S], BF16, kind="ExternalInput")
    wT = nc.dram_tensor("wT", [E, V3], BF16, kind="ExternalInput")
    bq = nc.dram_tensor("bq", [128, 4], F32, kind="ExternalInput")
    bv = nc.dram_tensor("bv", [1, 256], BF16, kind="ExternalInput")
    woT = nc.dram_tensor("woT", [2 * 128, E], BF16, kind="ExternalInput")
    ident = nc.dram_tensor("ident", [128, 128], F32, kind="ExternalInput")
    out0 = nc.dram_tensor("out0", [S, E], BF16, kind="ExternalOutput")
    out1 = nc.dram_tensor("out1", [S, E], BF16, kind="ExternalOutput")
    outd = {0: out0, 1: out1}

    with tile.TileContext(nc) as tc:
        with (
            tc.tile_pool(name="persist", bufs=1) as persist,
            tc.tile_pool(name="smalls", bufs=4) as smalls,
            tc.tile_pool(name="expp", bufs=2 * FG) as expp,
            tc.tile_pool(name="vsp", bufs=2 * FG + 2) as vsp,
            tc.tile_pool(name="fout", bufs=4) as foutp,
            tc.tile_pool(name="mm_ps", bufs=2, space="PSUM") as mm_ps,
            tc.tile_pool(name="sp_ps", bufs=2, space="PSUM") as sp_ps,
            tc.tile_pool(name="ot_ps", bufs=2, space="PSUM") as ot_ps,
        ):
            qk_sb = persist.tile([128, 4, S], BF16, tag="qk")
            v_sb = persist.tile([128, ST, 256], F32, tag="v")
            out_sb = persist.tile([128, 2, ST, 128], F32, tag="out")
            outT_bf = persist.tile([128, 2, S], BF16, tag="outT")
            bq_sb = persist.tile([128, 4], F32, tag="bq")
            bv_sb = persist.tile([1, 256], BF16, tag="bv")
            ones_sb = persist.tile([1, 128], BF16, tag="ones")
            id_sb = persist.tile([128, 128], F32, tag="ident")
            xt_sb = persist.tile([128, ET, S], BF16, tag="xt")
            wt_sb = persist.tile([128, ET, V3], BF16, tag="wt")
            wo_sb = persist.tile([128, 2, E], BF16, tag="wo")

            nc.vector.memset(ones_sb[:], 1.0)

            def dma_xt(sc, et):
                nc.gpsimd.dma_start(
                    xt_sb[:, et, sc * 512 : (sc + 1) * 512],
                    xT[et * 128 : (et + 1) * 128, sc * 512 : (sc + 1) * 512],
                )

            # dma_start occupies the ISSUING engine for ~the transfer time,
            # so spread input DMAs across gpsimd/SP/ACT: the critical
            # wt/sc0/sc1 chunks land in parallel within ~5us. ACT's batch
            # (sc1) sits before its exp stream and finishes by ~4.5us.
            nc.sync.dma_start(bq_sb[:], bq[:])
            for et in range(ET):
                nc.gpsimd.dma_start(wt_sb[:, et, :], wT[et * 128 : (et + 1) * 128, :])
            for et in range(ET):
                nc.sync.dma_start(
                    xt_sb[:, et, 0:512], xT[et * 128 : (et + 1) * 128, 0:512]
                )
            for et in range(ET):
                nc.scalar.dma_start(
                    xt_sb[:, et, 512:1024], xT[et * 128 : (et + 1) * 128, 512:1024]
                )
            for et in range(5):
                nc.scalar.dma_start(
                    xt_sb[:, et, 1024:1536], xT[et * 128 : (et + 1) * 128, 1024:1536]
                )
            for et in range(5):
                nc.sync.dma_start(
                    xt_sb[:, et, 1536:2048], xT[et * 128 : (et + 1) * 128, 1536:2048]
                )
            for et in range(5, ET):
                nc.gpsimd.dma_start(
                    xt_sb[:, et, 1024:1536], xT[et * 128 : (et + 1) * 128, 1024:1536]
                )
                nc.gpsimd.dma_start(
                    xt_sb[:, et, 1536:2048], xT[et * 128 : (et + 1) * 128, 1536:2048]
                )
            nc.sync.dma_start(bv_sb[:], bv[:])
            nc.sync.dma_start(id_sb[:], ident[:])
            for p in range(2):
                nc.sync.dma_start(wo_sb[:, p, :], woT[p * 128 : (p + 1) * 128, :])

            # ---- qkv projection group emitters -------------------------
            # (GPSIMD cannot touch PSUM on HW: bias-adds/copies go to DVE)
            def emit_qk_group(eo, sc, bias_dve=True):
                pt = mm_ps.tile([128, 512], F32, tag="mmps")
                for et in range(ET):
                    nc.tensor.matmul(
                        pt[:],
                        wt_sb[:, et, eo * 128 : (eo + 1) * 128],
                        xt_sb[:, et, sc * 512 : (sc + 1) * 512],
                        start=(et == 0),
                        stop=(et == ET - 1),
                    )
                nc.vector.tensor_scalar_add(
                    qk_sb[:, eo, sc * 512 : (sc + 1) * 512],
                    in0=pt[:],
                    scalar1=bq_sb[:, eo : eo + 1],
                )

            def emit_v_group(st, p):
                # half V-projection: this pair's 128 v-dims only. pair1's
                # groups borrow the otps psum tag (mm is busy with pair0 proj)
                v0 = QK + p * 128
                if p == 1:
                    ptt = ot_ps.tile([128, NQUAD, 128], F32, tag="otps")
                    pt = ptt[:, 0, :]
                else:
                    ptt = mm_ps.tile([128, 512], F32, tag="mmps")
                    pt = ptt[:, 0:128]
                for et in range(ET):
                    nc.tensor.matmul(
                        pt,
                        xt_sb[:, et, st * 128 : (st + 1) * 128],
                        wt_sb[:, et, v0 : v0 + 128],
                        start=(et == 0),
                        stop=False,
                    )
                nc.tensor.matmul(  # + ones^T bv (bias row)
                    pt,
                    ones_sb[0:1, :],
                    bv_sb[0:1, p * 128 : (p + 1) * 128],
                    start=False,
                    stop=True,
                )
                nc.vector.tensor_copy(v_sb[:, st, p * 128 : (p + 1) * 128], pt)

            def qg(eo, sc):
                return lambda: emit_qk_group(eo, sc)

            def vg(st, p=0):
                return lambda: emit_v_group(st, p)

            # ---- attn.V quad (4 qt's of one group) ---------------------
            GROUPS = [(0, 4), (4, 8), (8, 12), (12, 16)]

            def emit_quad(p, g, o, exs, vss, per_qt_drain=False):
                k0, k1 = GROUPS[g]
                ot = ot_ps.tile([128, NQUAD, 128], F32, tag="otps")
                for qi in range(NQUAD):
                    qt = NQUAD * o + qi
                    for hh in range(2):
                        for kt in range(k0, k1):
                            nc.tensor.matmul(
                                ot[:, qi, hh * 64 : (hh + 1) * 64],
                                exs[kt][:, hh, qt * 128 : (qt + 1) * 128],
                                vss[kt][:, hh, :],
                                start=(kt == k0),
                                stop=(kt == k1 - 1),
                            )
                    if per_qt_drain:
                        d = out_sb[:, p, qt : qt + 1, :]
                        nc.vector.tensor_add(d, d, ot[:, qi : qi + 1, :])
                if per_qt_drain:
                    return
                dst = out_sb[:, p, NQUAD * o : NQUAD * (o + 1), :]
                if g == 0:
                    nc.vector.tensor_copy(dst, ot[:])
                else:
                    nc.vector.tensor_add(dst, dst, ot[:])

            # ---- transpose block (4 qt's -> outT columns) --------------
            def emit_tblock(p, blk, tail=False):
                # f32 PE transpose straight from out_sb; the psum->sbuf copy
                # does the bf16 conversion (DVE; half on ACT at the tail)
                tp = ot_ps.tile([128, NQUAD, 128], F32, tag="otps")
                for qi in range(NQUAD):
                    nc.tensor.transpose(
                        tp[:, qi, :],
                        out_sb[:, p, NQUAD * blk + qi, :],
                        id_sb[:],
                    )
                for qi in range(NQUAD):
                    qt = NQUAD * blk + qi
                    eng = nc.scalar.copy if (tail and qi % 2 == 0) else (
                        lambda o, i: nc.vector.tensor_copy(o, i)
                    )
                    eng(
                        outT_bf[:, p, qt * 128 : (qt + 1) * 128],
                        tp[:, qi, :],
                    )

            # ---- output projection group (one s-tile, one pair) --------
            def emit_d_group(p, st, tail):
                if not tail:
                    # pair0 proj during pair1's loop: mm psum, copies on DVE
                    # (GPSIMD can't read PSUM), DMA on the idle SP
                    ot = foutp.tile([128, E], BF16, tag="fout", name=f"fo_{p}_{st}")
                    for nck in range(2):
                        pt = mm_ps.tile(
                            [128, 512], F32, tag="mmps", name=f"fp_{p}_{st}_{nck}"
                        )
                        nc.tensor.matmul(
                            pt[:],
                            outT_bf[:, p, st * 128 : (st + 1) * 128],
                            wo_sb[:, p, nck * 512 : (nck + 1) * 512],
                            start=True,
                            stop=True,
                        )
                        nc.vector.tensor_copy(
                            ot[:, nck * 512 : (nck + 1) * 512], pt[:]
                        )
                    nc.sync.dma_start(outd[p][st * 128 : (st + 1) * 128, :], ot[:])
                    return
                # tail (out1, bf16): psum alternates the freed scores slots
                # and the mm slots; both copies of a group go to ACT or DVE
                ot = foutp.tile([128, E], BF16, tag="fout", name=f"fo_{p}_{st}")
                if st % 3 == 2:
                    pts = [
                        mm_ps.tile([128, 512], F32, tag="mmps", name=f"fp_{st}_{n}")
                        for n in range(2)
                    ]
                    chunks = [pts[0][:], pts[1][:]]
                else:
                    pt = sp_ps.tile([128, 1024], F32, tag="sp", name=f"fp_{st}")
                    chunks = [pt[:, 0:512], pt[:, 512:1024]]
                for nck in range(2):
                    nc.tensor.matmul(
                        chunks[nck],
                        outT_bf[:, p, st * 128 : (st + 1) * 128],
                        wo_sb[:, p, nck * 512 : (nck + 1) * 512],
                        start=True,
                        stop=True,
                    )
                if st % 3 != 2 and st != KT - 1:
                    # contiguous psum: single wide copy; ACT takes ~7 of 11
                    # (DVE also carries the drains and transpose copies)
                    if st in (1, 6, 10, 13):
                        nc.vector.tensor_copy(ot[:], pt[:])
                    else:
                        nc.scalar.copy(ot[:], pt[:])
                else:
                    # mm-pair groups + the final group: split across engines
                    for nck in range(2):
                        eng = nc.scalar.copy if nck == 0 else (
                            lambda o, i: nc.vector.tensor_copy(o, i)
                        )
                        eng(ot[:, nck * 512 : (nck + 1) * 512], chunks[nck])
                nc.sync.dma_start(outd[p][st * 128 : (st + 1) * 128, :], ot[:])

            # ---- pre-attention: just enough for pair0 kt0 --------------
            # only what the FIRST 512-wide exp needs; Q-sc1 is emitted
            # between the two split chunks inside kt0
            emit_qk_group(0, 0)  # Q pair0, q cols 0-511
            emit_qk_group(2, 0)  # K pair0, k tiles 0-3


            # pair0 fillers (popped per-kt by pattern below). V-projection is
            # split per pair: pair0's halves here, pair1's in its own loop.
            fillers0 = [
                vg(0), vg(1), vg(2), qg(2, 1), vg(3), vg(4), vg(5), qg(2, 2),
                vg(6), vg(7), vg(8), qg(2, 3), vg(9), vg(10), vg(11), qg(3, 0),
                vg(12), vg(13), vg(14), vg(15),
                qg(1, 0), qg(1, 1), qg(1, 2), qg(1, 3),
            ]
            fillers0.reverse()
            pops0 = [1, 3, 3, 2, 2, 2, 2, 2, 1, 1, 1, 1, 1, 1, 1, 0]  # = 24

            # pair1 in-loop fillers: its V halves + leftover K groups
            fillers1 = [
                vg(0, 1), vg(1, 1), qg(3, 1), vg(2, 1), vg(3, 1), vg(4, 1),
                qg(3, 2), vg(5, 1), vg(6, 1), vg(7, 1), qg(3, 3), vg(8, 1),
                vg(9, 1), vg(10, 1), vg(11, 1), vg(12, 1), vg(13, 1),
                vg(14, 1), vg(15, 1),
            ]
            fillers1.reverse()
            pops1 = [2, 2, 2, 2, 1, 2, 1, 2, 1, 1, 1, 1, 1, 0, 0, 0]  # = 19
            # pair0 proj schedule over pair1 kts 3..15
            proj0 = [2, 1, 1, 1, 1, 1, 1, 1, 1, 2, 1, 2, 1]  # sums to 16

            exs_p0 = vss_p0 = None
            for p in range(2):
                exs = {}
                vss = {}
                proj_done = 0
                def emit_chunk(kt, ex, den, half, hh):
                    sp = sp_ps.tile([128, 1024], F32, tag="sp", name="sp")
                    for qc in range(2):
                        q0 = half * 1024 + qc * 512
                        nc.tensor.matmul(
                            sp[:, qc * 512 : (qc + 1) * 512],
                            qk_sb[
                                hh * 64 : (hh + 1) * 64, 2 + p, kt * 128 : (kt + 1) * 128
                            ],
                            qk_sb[hh * 64 : (hh + 1) * 64, p, q0 : q0 + 512],
                            start=True,
                            stop=True,
                        )
                    exc = ex[:, hh, half * 1024 : (half + 1) * 1024]
                    nc.scalar.activation(exc, sp[:], AF.Exp, scale=0.125)
                    nc.vector.tensor_scalar(
                        out=exc,
                        in0=exc,
                        scalar1=1.0,
                        scalar2=None,
                        op0=mybir.AluOpType.mult,
                        op1=mybir.AluOpType.add,
                        accum_out=den[:, hh, half : half + 1],
                    )

                den2_kt0 = None
                if p == 0:
                    # hand-rolled kts 0-1: interleave both k-tiles' half0
                    # chunks with the remaining Q-group emissions so ACT
                    # never runs dry while PE grinds through qk01/02/03
                    ex0 = expp.tile([128, 2, S], BF16, tag="exp", name="ex0")
                    exs[0] = ex0
                    den0 = smalls.tile([128, 2, 2], F32, tag="den")
                    den2_kt0 = smalls.tile([128, 1], F32, tag="den2")
                    sp = sp_ps.tile([128, 1024], F32, tag="sp", name="sp00")
                    for qc in range(2):
                        nc.tensor.matmul(
                            sp[:, qc * 512 : (qc + 1) * 512],
                            qk_sb[0:64, 2, 0:128],
                            qk_sb[0:64, 0, qc * 512 : (qc + 1) * 512],
                            start=True,
                            stop=True,
                        )
                        dst = den0[:, 0, 0:1] if qc == 0 else den2_kt0[:]
                        nc.scalar.activation(
                            ex0[:, 0, qc * 512 : (qc + 1) * 512],
                            sp[:, qc * 512 : (qc + 1) * 512],
                            AF.Exp,
                            scale=0.125,
                            accum_out=dst,
                        )
                        if qc == 0:
                            emit_qk_group(0, 1)  # Q cols 512-1023
                    emit_chunk(0, ex0, den0, 0, 1)
                    warm = [(0, ex0, den0)]
                    for kt_ in (1, 2, 3):
                        ex_ = expp.tile([128, 2, S], BF16, tag="exp", name=f"exw{kt_}")
                        exs[kt_] = ex_
                        den_ = smalls.tile([128, 2, 2], F32, tag="den")
                        warm.append((kt_, ex_, den_))
                        emit_chunk(kt_, ex_, den_, 0, 0)
                        if kt_ == 1:
                            emit_qk_group(0, 2)  # Q cols 1024-1535
                        emit_chunk(kt_, ex_, den_, 0, 1)
                        if kt_ == 1:
                            emit_qk_group(0, 3)  # Q cols 1536-2047
                    for kt_, ex_, den_ in warm:
                        for hh in range(2):
                            emit_chunk(kt_, ex_, den_, 1, hh)
                    for _ in range(sum(pops0[0:4])):
                        if fillers0:
                            fillers0.pop()()
                    for kt_, ex_, den_ in warm:
                        dsum = smalls.tile([128, 2], F32, tag="dsum")
                        nc.gpsimd.tensor_add(
                            dsum[:], den_[:, :, 0], den_[:, :, 1]
                        )
                        if kt_ == 0:
                            nc.gpsimd.tensor_add(
                                dsum[:, 0:1], dsum[:, 0:1], den2_kt0[:]
                            )
                        rec = smalls.tile([128, 2], F32, tag="rec")
                        nc.vector.reciprocal(rec[:], dsum[:])
                        vs = vsp.tile([128, 2, DH], BF16, tag="vs")
                        vss[kt_] = vs
                        for hh in range(2):
                            nc.gpsimd.tensor_scalar_mul(
                                vs[:, hh, :],
                                in0=v_sb[
                                    :, kt_, (2 * p + hh) * 64 : (2 * p + hh + 1) * 64
                                ],
                                scalar1=rec[:, hh : hh + 1],
                            )

                for kt in range(4 if p == 0 else 0, KT):
                    ex = expp.tile([128, 2, S], BF16, tag="exp")
                    exs[kt] = ex
                    den = smalls.tile([128, 2, 2], F32, tag="den")
                    den2 = None
                    for half in range(2):
                        for hh in range(2):
                            sp = sp_ps.tile([128, 1024], F32, tag="sp")
                            for qc in range(2):
                                q0 = half * 1024 + qc * 512
                                nc.tensor.matmul(
                                    sp[:, qc * 512 : (qc + 1) * 512],
                                    qk_sb[
                                        hh * 64 : (hh + 1) * 64,
                                        2 + p,
                                        kt * 128 : (kt + 1) * 128,
                                    ],
                                    qk_sb[hh * 64 : (hh + 1) * 64, p, q0 : q0 + 512],
                                    start=True,
                                    stop=True,
                                )
                            exc = ex[:, hh, half * 1024 : (half + 1) * 1024]
                            dslice = den[:, hh, half : half + 1]
                            # offload softmax-denominator accumulation to DVE
                            # for ~40 of each pair's 64 chunks (ACT/DVE
                            # balance); keep the final k-tile on ACT so the
                            # tail's first quad isn't gated on a DVE reduce
                            # denominators via DVE tensor_scalar(mult 1.0)
                            # with fused accum: 4x DVE mode makes this ~330ns
                            # so ACT keeps only the exps. Final k-tile stays
                            # on ACT accum (shortest path into the tail).
                            offload = True
                            if offload:
                                nc.scalar.activation(exc, sp[:], AF.Exp, scale=0.125)
                                nc.vector.tensor_scalar(
                                    out=exc,
                                    in0=exc,
                                    scalar1=1.0,
                                    scalar2=None,
                                    op0=mybir.AluOpType.mult,
                                    op1=mybir.AluOpType.add,
                                    accum_out=dslice,
                                )
                            else:
                                nc.scalar.activation(
                                    exc, sp[:], AF.Exp, scale=0.125, accum_out=dslice
                                )
                        if kt == 0 and half == 0 and hh == 1 and p == 0:
                            # Q cols 1024-2047 for pair0 (needed by half1);
                            # pair1's Q groups all complete during pair0.
                            emit_qk_group(0, 2)
                            emit_qk_group(0, 3)

                    # attn.V quad of the previous group (or pair0 spill)
                    if kt >= FG:
                        g, o = kt // FG - 1, kt % FG
                        emit_quad(p, g, o, exs, vss)
                    elif p == 1:
                        emit_quad(0, 3, kt, exs_p0, vss_p0)

                    # fillers / pair0-tail / proj interleaves
                    if p == 0:
                        for _ in range(pops0[kt]):
                            if fillers0:
                                fillers0.pop()()
                    else:
                        for _ in range(pops1[kt]):
                            if fillers1:
                                fillers1.pop()()
                        if 2 <= kt <= 5:
                            emit_tblock(0, kt - 2)
                        if kt >= 3:
                            for _ in range(proj0[kt - 3]):
                                if proj_done < NQUAD * (kt - 1):
                                    emit_d_group(0, proj_done, tail=False)
                                    proj_done += 1

                    # denominators -> 1/denom -> scaled V for this k-tile
                    # (combine/scale on Pool; reciprocal is DVE-only)
                    dsum = smalls.tile([128, 2], F32, tag="dsum")
                    nc.gpsimd.tensor_add(dsum[:], den[:, :, 0], den[:, :, 1])
                    if den2 is not None:
                        nc.gpsimd.tensor_add(dsum[:, 0:1], dsum[:, 0:1], den2[:])
                    rec = smalls.tile([128, 2], F32, tag="rec")
                    nc.vector.reciprocal(rec[:], dsum[:])
                    vs = vsp.tile([128, 2, DH], BF16, tag="vs")
                    vss[kt] = vs
                    for hh in range(2):
                        nc.gpsimd.tensor_scalar_mul(
                            vs[:, hh, :],
                            in0=v_sb[:, kt, (2 * p + hh) * 64 : (2 * p + hh + 1) * 64],
                            scalar1=rec[:, hh : hh + 1],
                        )

                if p == 0:
                    exs_p0, vss_p0 = exs, vss
                else:
                    # tail: stay one quad ahead so PE never idles while a
                    # block's drain/convert hop across DVE/ACT
                    emit_quad(1, 3, 0, exs, vss)
                    for o in range(NQUAD):
                        emit_tblock(1, o, tail=True)
                        if o + 1 < NQUAD:
                            emit_quad(1, 3, o + 1, exs, vss)
                        for st in range(NQUAD * o, NQUAD * (o + 1)):
                            emit_d_group(1, st, tail=True)

    nc.compile()
    return nc


def _shard_inputs(input, Wqkv, bqkv, Wo):
    """Build the 8 per-core input dicts (host-side layout/sharding)."""
    bf16 = ml_dtypes.bfloat16
    ident_f32 = np.eye(128, dtype=np.float32)
    in_maps = []
    for c in range(NCORES):
        b = c // 4
        g = c % 4
        heads = range(4 * g, 4 * g + 4)
        rows = (
            [slice(64 * h, 64 * h + 64) for h in heads]
            + [slice(E + 64 * h, E + 64 * h + 64) for h in heads]
            + [slice(2 * E + 64 * h, 2 * E + 64 * h + 64) for h in heads]
        )
        W_sel = np.concatenate([Wqkv[s] for s in rows], axis=0)  # [768, 1024]
        b_sel = np.concatenate([bqkv[s] for s in rows], axis=0)  # [768]
        in_maps.append(
            {
                "xT": np.ascontiguousarray(input[b].T).astype(bf16),
                "wT": np.ascontiguousarray(W_sel.T).astype(bf16),
                "bq": np.ascontiguousarray(b_sel[:QK].reshape(4, 128).T),
                "bv": np.ascontiguousarray(b_sel[QK:V3].reshape(1, 256)).astype(bf16),
                "woT": np.ascontiguousarray(
                    Wo[:, 4 * g * DH : 4 * (g + 1) * DH].T
                ).astype(bf16),
                "ident": ident_f32,
            }
        )
    return in_maps


def kernel(input, Wqkv, bqkv, Wo, bo, _trace=False):
    global LAST_RESULTS
    input = np.asarray(input, dtype=np.float32)
    Wqkv = np.asarray(Wqkv, dtype=np.float32)
    bqkv = np.asarray(bqkv, dtype=np.float32)
    Wo = np.asarray(Wo, dtype=np.float32)
    bo = np.asarray(bo, dtype=np.float32)

    nc = build_kernel()
    in_maps = _shard_inputs(input, Wqkv, bqkv, Wo)
    kwargs = {}
    if _trace:
        kwargs = dict(trace=True, trace_cores=[0])
    try:
        res = run_bass_kernel_spmd(nc, in_maps, core_ids=list(range(NCORES)), **kwargs)
    except ModuleNotFoundError:
        # no NTFF profiling hook in this container — run without trace
        res = run_bass_kernel_spmd(nc, in_maps, core_ids=list(range(NCORES)))
    LAST_RESULTS = res

    out = np.zeros((B, S, E), dtype=np.float32)
    for c in range(NCORES):
        out[c // 4] += res.results[c]["out0"]
        out[c // 4] += res.results[c]["out1"]
    out += bo
    return out


# revision 16
# speedup vs baseline: 1.0225x; 1.0018x over previous
"""Multi-head attention (softmax over the QUERY axis) on 8 TRN2 NeuronCores.

Sharding: 2 batches x 4 head-groups (4 heads each) -> 8 cores; each core
processes its 4 heads as two head PAIRS (p=0,1).

Per (batch b, head pair p) on a core:
    qkT = W_{q,k} @ x_b^T + b_{q,k}        [512, 2048]  (e_out on partitions)
    V   = x_b @ W_v^T + b_v                [2048, 256]  (per-pair halves)
    S'  = K Q^T  (scores TRANSPOSED)       [k, q] per head
    P'  = exp(S'/8), denom[k] = sum_q P'   (ACT accum_out or DVE tensor_reduce)
    V'  = V[k,:]/denom[k]                  (scaled per k-tile)
    out[q, d] = sum_k P'[k,q] V'[k,d]      <-- P' stationary, V' moving (N=64)
    outT = f32 PE identity-transpose of out, bf16 on psum->sbuf copy
    part = outT^T @ WoT_p                  [2048, 1024] partial per pair, bf16
Host sums the 8 bf16 partials per batch (fp32 accumulate) and adds bo.

Design notes (vs the transposed-attn.V baseline):
- attn.V in out[q,d] form halves its PE cost (matmul cost ~ moving free
  size: N=64 instead of N=q=512 per stationary), PE ~140us vs ~165us.
- The exp stream is the bottleneck and runs on ACT alone (~134us,
  gapless). Softmax denominators come from DVE tensor_scalar(mult 1.0,
  op1=add, accum_out) on the bf16 P' in SBUF, which hits the 4x DVE
  mode (~330ns per [128,1024] chunk vs 1127ns for tensor_reduce); only
  the final k-tile keeps ACT accum_out (shortest path into the tail).
- GPSIMD must NOT touch PSUM (BIR verifier): every psum->sbuf evacuation
  (qk bias-adds, V copies, attn.V quad drains, transpose copies, proj
  copies) is on DVE/ACT; Pool only does DMA issue, dsum, and V-scaling.
- dma_start occupies the issuing engine ~transfer-time; input DMAs are
  spread over Pool/SP/ACT, outputs ride SP; outputs are bf16.
- PSUM (8 banks): scores 2x[128,1024]f32 + quads/transposes/pair1-V
  2x[128,4,128]f32 + qkv/proj 2x[128,512]f32.
- Schedule: pair0's qkv fillers keep PE ~balanced with ACT; pair1's loop
  carries pair0's tail (last attn.V group, transposes, projection); the
  final tail pipelines quad->transpose->proj->DMA per 4-qt block.
"""

import sys

if "/opt/trn_rl_repo" not in sys.path:
    sys.path.insert(0, "/opt/trn_rl_repo")

import numpy as np
import ml_dtypes

import concourse.bass as bass
import concourse.mybir as mybir
import concourse.tile as tile
from concourse import bacc
from concourse.bass_utils import run_bass_kernel_spmd

F32 = mybir.dt.float32
BF16 = mybir.dt.bfloat16
AF = mybir.ActivationFunctionType

B, S, E, H = 2, 2048, 1024, 16
HL = 4  # heads per core
DH = 64
QK = 512
V3 = 768
NCORES = 8

ET = E // 128  # 8
ST = S // 128  # 16
SC = S // 512  # 4
KT = ST
FG = 4  # k-tiles per attn.V group
NQUAD = 4  # qt's per attn.V psum quad

LAST_RESULTS = None


def build_kernel():
    nc = bacc.Bacc("TRN2", target_bir_lowering=False, debug=False, num_devices=NCORES)

    xT = nc.dram_tensor("xT", [E, S], BF16, kind="ExternalInput")
    wT = nc.dram_tensor("wT", [E, V3], BF16, kind="ExternalInput")
    bq = nc.dram_tensor("bq", [128, 4], F32, kind="ExternalInput")
    bv = nc.dram_tensor("bv", [1, 256], BF16, kind="ExternalInput")
    woT = nc.dram_tensor("woT", [2 * 128, E], BF16, kind="ExternalInput")
    ident = nc.dram_tensor("ident", [128, 128], F32, kind="ExternalInput")
    out0 = nc.dram_tensor("out0", [S, E], BF16, kind="ExternalOutput")
    out1 = nc.dram_tensor("out1", [S, E], BF16, kind="ExternalOutput")
    outd = {0: out0, 1: out1}

    with tile.TileContext(nc) as tc:
        with (
            tc.tile_pool(name="persist", bufs=1) as persist,
            tc.tile_pool(name="smalls", bufs=4) as smalls,
            tc.tile_pool(name="expp", bufs=2 * FG) as expp,
            tc.tile_pool(name="vsp", bufs=2 * FG + 2) as vsp,
            tc.tile_pool(name="fout", bufs=4) as foutp,
            tc.tile_pool(name="mm_ps", bufs=2, space="PSUM") as mm_ps,
            tc.tile_pool(name="sp_ps", bufs=2, space="PSUM") as sp_ps,
            tc.tile_pool(name="ot_ps", bufs=2, space="PSUM") as ot_ps,
        ):
            qk_sb = persist.tile([128, 4, S], BF16, tag="qk")
            v_sb = persist.tile([128, ST, 256], F32, tag="v")
            out_sb = persist.tile([128, 2, ST, 128], F32, tag="out")
            outT_bf = persist.tile([128, 2, S], BF16, tag="outT")
            bq_sb = persist.tile([128, 4], F32, tag="bq")
            bv_sb = persist.tile([1, 256], BF16, tag="bv")
            ones_sb = persist.tile([1, 128], BF16, tag="ones")
            id_sb = persist.tile([128, 128], F32, tag="ident")
            xt_sb = persist.tile([128, ET, S], BF16, tag="xt")
            wt_sb = persist.tile([128, ET, V3], BF16, tag="wt")
            wo_sb = persist.tile([128, 2, E], BF16, tag="wo")

            nc.vector.memset(ones_sb[:], 1.0)

            def dma_xt(sc, et):
                nc.gpsimd.dma_start(
                    xt_sb[:, et, sc * 512 : (sc + 1) * 512],
                    xT[et * 128 : (et + 1) * 128, sc * 512 : (sc + 1) * 512],
                )

            # dma_start occupies the ISSUING engine for ~the transfer time,
            # so spread input DMAs across gpsimd/SP/ACT: the critical
            # wt/sc0/sc1 chunks land in parallel within ~5us. ACT's batch
            # (sc1) sits before its exp stream and finishes by ~4.5us.
            nc.sync.dma_start(bq_sb[:], bq[:])
            for et in range(ET):
                nc.gpsimd.dma_start(wt_sb[:, et, :], wT[et * 128 : (et + 1) * 128, :])
            for et in range(ET):
                nc.sync.dma_start(
                    xt_sb[:, et, 0:512], xT[et * 128 : (et + 1) * 128, 0:512]
                )
            for et in range(ET):
                nc.scalar.dma_start(
                    xt_sb[:, et, 512:1024], xT[et * 128 : (et + 1) * 128, 512:1024]
                )
            for et in range(5):
                nc.scalar.dma_start(
                    xt_sb[:, et, 1024:1536], xT[et * 128 : (et + 1) * 128, 1024:1536]
                )
            for et in range(5):
                nc.sync.dma_start(
                    xt_sb[:, et, 1536:2048], xT[et * 128 : (et + 1) * 128, 1536:2048]
                )
            for et in range(5, ET):
                nc.gpsimd.dma_start(
                    xt_sb[:, et, 1024:1536], xT[et * 128 : (et + 1) * 128, 1024:1536]
                )
                nc.gpsimd.dma_start(
                    xt_sb[:, et, 1536:2048], xT[et * 128 : (et + 1) * 128, 1536:2048]
                )
            nc.sync.dma_start(bv_sb[:], bv[:])
            nc.sync.dma_start(id_sb[:], ident[:])
            for p in range(2):
                nc.sync.dma_start(wo_sb[:, p, :], woT[p * 128 : (p + 1) * 128, :])

            # ---- qkv projection group emitters -------------------------
            # (GPSIMD cannot touch PSUM on HW: bias-adds/copies go to DVE)
            def emit_qk_group(eo, sc, bias_dve=True):
                pt = mm_ps.tile([128, 512], F32, tag="mmps")
                for et in range(ET):
                    nc.tensor.matmul(
                        pt[:],
                        wt_sb[:, et, eo * 128 : (eo + 1) * 128],
                        xt_sb[:, et, sc * 512 : (sc + 1) * 512],
                        start=(et == 0),
                        stop=(et == ET - 1),
                    )
                nc.vector.tensor_scalar_add(
                    qk_sb[:, eo, sc * 512 : (sc + 1) * 512],
                    in0=pt[:],
                    scalar1=bq_sb[:, eo : eo + 1],
                )

            def emit_v_group(st, p):
                # half V-projection: this pair's 128 v-dims only. pair1's
                # groups borrow the otps psum tag (mm is busy with pair0 proj)
                v0 = QK + p * 128
                if p == 1:
                    ptt = ot_ps.tile([128, NQUAD, 128], F32, tag="otps")
                    pt = ptt[:, 0, :]
                else:
                    ptt = mm_ps.tile([128, 512], F32, tag="mmps")
                    pt = ptt[:, 0:128]
                for et in range(ET):
                    nc.tensor.matmul(
                        pt,
                        xt_sb[:, et, st * 128 : (st + 1) * 128],
                        wt_sb[:, et, v0 : v0 + 128],
                        start=(et == 0),
                        stop=False,
                    )
                nc.tensor.matmul(  # + ones^T bv (bias row)
                    pt,
                    ones_sb[0:1, :],
                    bv_sb[0:1, p * 128 : (p + 1) * 128],
                    start=False,
                    stop=True,
                )
                nc.vector.tensor_copy(v_sb[:, st, p * 128 : (p + 1) * 128], pt)

            def qg(eo, sc):
                return lambda: emit_qk_group(eo, sc)

            def vg(st, p=0):
                return lambda: emit_v_group(st, p)

            # ---- attn.V quad (4 qt's of one group) ---------------------
            GROUPS = [(0, 4), (4, 8), (8, 12), (12, 16)]

            def emit_quad(p, g, o, exs, vss, per_qt_drain=False):
                k0, k1 = GROUPS[g]
                ot = ot_ps.tile([128, NQUAD, 128], F32, tag="otps")
                for qi in range(NQUAD):
                    qt = NQUAD * o + qi
                    for hh in range(2):
                        for kt in range(k0, k1):
                            nc.tensor.matmul(
                                ot[:, qi, hh * 64 : (hh + 1) * 64],
                                exs[kt][:, hh, qt * 128 : (qt + 1) * 128],
                                vss[kt][:, hh, :],
                                start=(kt == k0),
                                stop=(kt == k1 - 1),
                            )
                    if per_qt_drain:
                        d = out_sb[:, p, qt : qt + 1, :]
                        nc.vector.tensor_add(d, d, ot[:, qi : qi + 1, :])
                if per_qt_drain:
                    return
                dst = out_sb[:, p, NQUAD * o : NQUAD * (o + 1), :]
                if g == 0:
                    nc.vector.tensor_copy(dst, ot[:])
                else:
                    nc.vector.tensor_add(dst, dst, ot[:])

            # ---- transpose block (4 qt's -> outT columns) --------------
            def emit_tblock(p, blk, tail=False):
                # f32 PE transpose straight from out_sb; the psum->sbuf copy
                # does the bf16 conversion (DVE; half on ACT at the tail)
                tp = ot_ps.tile([128, NQUAD, 128], F32, tag="otps")
                for qi in range(NQUAD):
                    nc.tensor.transpose(
                        tp[:, qi, :],
                        out_sb[:, p, NQUAD * blk + qi, :],
                        id_sb[:],
                    )
                for qi in range(NQUAD):
                    qt = NQUAD * blk + qi
                    eng = nc.scalar.copy if (tail and qi % 2 == 0) else (
                        lambda o, i: nc.vector.tensor_copy(o, i)
                    )
                    eng(
                        outT_bf[:, p, qt * 128 : (qt + 1) * 128],
                        tp[:, qi, :],
                    )

            # ---- output projection group (one s-tile, one pair) --------
            def emit_d_group(p, st, tail):
                if not tail:
                    # pair0 proj during pair1's loop: mm psum, copies on DVE
                    # (GPSIMD can't read PSUM), DMA on the idle SP
                    ot = foutp.tile([128, E], BF16, tag="fout", name=f"fo_{p}_{st}")
                    for nck in range(2):
                        pt = mm_ps.tile(
                            [128, 512], F32, tag="mmps", name=f"fp_{p}_{st}_{nck}"
                        )
                        nc.tensor.matmul(
                            pt[:],
                            outT_bf[:, p, st * 128 : (st + 1) * 128],
                            wo_sb[:, p, nck * 512 : (nck + 1) * 512],
                            start=True,
                            stop=True,
                        )
                        nc.vector.tensor_copy(
                            ot[:, nck * 512 : (nck + 1) * 512], pt[:]
                        )
                    nc.sync.dma_start(outd[p][st * 128 : (st + 1) * 128, :], ot[:])
                    return
                # tail (out1, bf16): psum alternates the freed scores slots
                # and the mm slots; both copies of a group go to ACT or DVE
                ot = foutp.tile([128, E], BF16, tag="fout", name=f"fo_{p}_{st}")
                if st % 3 == 2:
                    pts = [
                        mm_ps.tile([128, 512], F32, tag="mmps", name=f"fp_{st}_{n}")
                        for n in range(2)
                    ]
                    chunks = [pts[0][:], pts[1][:]]
                else:
                    pt = sp_ps.tile([128, 1024], F32, tag="sp", name=f"fp_{st}")
                    chunks = [pt[:, 0:512], pt[:, 512:1024]]
                for nck in range(2):
                    nc.tensor.matmul(
                        chunks[nck],
                        outT_bf[:, p, st * 128 : (st + 1) * 128],
                        wo_sb[:, p, nck * 512 : (nck + 1) * 512],
                        start=True,
                        stop=True,
                    )
                if st % 3 != 2 and st != KT - 1:
                    # contiguous psum: single wide copy; ACT takes ~7 of 11
                    # (DVE also carries the drains and transpose copies)
                    if st in (1, 6, 10, 13):
                        nc.vector.tensor_copy(ot[:], pt[:])
                    else:
                        nc.scalar.copy(ot[:], pt[:])
                else:
                    # mm-pair groups + the final group: split across engines
                    for nck in range(2):
                        eng = nc.scalar.copy if nck == 0 else (
                            lambda o, i: nc.vector.tensor_copy(o, i)
                        )
                        eng(ot[:, nck * 512 : (nck + 1) * 512], chunks[nck])
                nc.sync.dma_start(outd[p][st * 128 : (st + 1) * 128, :], ot[:])

            # ---- pre-attention: just enough for pair0 kt0 --------------
            # only what the FIRST 512-wide exp needs; Q-sc1 is emitted
            # between the two split chunks inside kt0
            emit_qk_group(0, 0)  # Q pair0, q cols 0-511
            emit_qk_group(2, 0)  # K pair0, k tiles 0-3


            # pair0 fillers (popped per-kt by pattern below). V-projection is
            # split per pair: pair0's halves here, pair1's in its own loop.
            fillers0 = [
                vg(0), vg(1), vg(2), qg(2, 1), vg(3), vg(4), vg(5), qg(2, 2),
                vg(6), vg(7), vg(8), qg(2, 3), vg(9), vg(10), vg(11), qg(3, 0),
                vg(12), vg(13), vg(14), vg(15),
                qg(1, 0), qg(1, 1), qg(1, 2), qg(1, 3),
            ]
            fillers0.reverse()
            pops0 = [1, 3, 3, 2, 2, 2, 2, 2, 1, 1, 1, 1, 1, 1, 1, 0]  # = 24

            # pair1 in-loop fillers: its V halves + leftover K groups
            fillers1 = [
                vg(0, 1), vg(1, 1), qg(3, 1), vg(2, 1), vg(3, 1), vg(4, 1),
                qg(3, 2), vg(5, 1), vg(6, 1), vg(7, 1), qg(3, 3), vg(8, 1),
                vg(9, 1), vg(10, 1), vg(11, 1), vg(12, 1), vg(13, 1),
                vg(14, 1), vg(15, 1),
            ]
            fillers1.reverse()
            pops1 = [2, 2, 2, 2, 1, 2, 1, 2, 1, 1, 1, 1, 1, 0, 0, 0]  # = 19
            # pair0 proj schedule over pair1 kts 3..15
            proj0 = [2, 1, 1, 1, 1, 1, 1, 1, 1, 2, 1, 2, 1]  # sums to 16

            exs_p0 = vss_p0 = None
            for p in range(2):
                exs = {}
                vss = {}
                proj_done = 0
                def emit_chunk(kt, ex, den, half, hh):
                    sp = sp_ps.tile([128, 1024], F32, tag="sp", name="sp")
                    for qc in range(2):
                        q0 = half * 1024 + qc * 512
                        nc.tensor.matmul(
                            sp[:, qc * 512 : (qc + 1) * 512],
                            qk_sb[
                                hh * 64 : (hh + 1) * 64, 2 + p, kt * 128 : (kt + 1) * 128
                            ],
                            qk_sb[hh * 64 : (hh + 1) * 64, p, q0 : q0 + 512],
                            start=True,
                            stop=True,
                        )
                    exc = ex[:, hh, half * 1024 : (half + 1) * 1024]
                    nc.scalar.activation(exc, sp[:], AF.Exp, scale=0.125)
                    nc.vector.tensor_scalar(
                        out=exc,
                        in0=exc,
                        scalar1=1.0,
                        scalar2=None,
                        op0=mybir.AluOpType.mult,
                        op1=mybir.AluOpType.add,
                        accum_out=den[:, hh, half : half + 1],
                    )

                den2_kt0 = None
                if p == 0:
                    # hand-rolled kts 0-1: interleave both k-tiles' half0
                    # chunks with the remaining Q-group emissions so ACT
                    # never runs dry while PE grinds through qk01/02/03
                    ex0 = expp.tile([128, 2, S], BF16, tag="exp", name="ex0")
                    exs[0] = ex0
                    den0 = smalls.tile([128, 2, 2], F32, tag="den")
                    den2_kt0 = smalls.tile([128, 1], F32, tag="den2")
                    sp = sp_ps.tile([128, 1024], F32, tag="sp", name="sp00")
                    for qc in range(2):
                        nc.tensor.matmul(
                            sp[:, qc * 512 : (qc + 1) * 512],
                            qk_sb[0:64, 2, 0:128],
                            qk_sb[0:64, 0, qc * 512 : (qc + 1) * 512],
                            start=True,
                            stop=True,
                        )
                        dst = den0[:, 0, 0:1] if qc == 0 else den2_kt0[:]
                        nc.scalar.activation(
                            ex0[:, 0, qc * 512 : (qc + 1) * 512],
                            sp[:, qc * 512 : (qc + 1) * 512],
                            AF.Exp,
                            scale=0.125,
                            accum_out=dst,
                        )
                        if qc == 0:
                            emit_qk_group(0, 1)  # Q cols 512-1023
                    emit_chunk(0, ex0, den0, 0, 1)
                    warm = [(0, ex0, den0)]
                    for kt_ in (1, 2, 3):
                        ex_ = expp.tile([128, 2, S], BF16, tag="exp", name=f"exw{kt_}")
                        exs[kt_] = ex_
                        den_ = smalls.tile([128, 2, 2], F32, tag="den")
                        warm.append((kt_, ex_, den_))
                        emit_chunk(kt_, ex_, den_, 0, 0)
                        if kt_ == 1:
                            emit_qk_group(0, 2)  # Q cols 1024-1535
                        emit_chunk(kt_, ex_, den_, 0, 1)
                        if kt_ == 1:
                            emit_qk_group(0, 3)  # Q cols 1536-2047
                    for kt_, ex_, den_ in warm:
                        for hh in range(2):
                            emit_chunk(kt_, ex_, den_, 1, hh)
                    for _ in range(sum(pops0[0:4])):
                        if fillers0:
                            fillers0.pop()()
                    for kt_, ex_, den_ in warm:
                        dsum = smalls.tile([128, 2], F32, tag="dsum")
                        nc.gpsimd.tensor_add(
                            dsum[:], den_[:, :, 0], den_[:, :, 1]
                        )
                        if kt_ == 0:
                            nc.gpsimd.tensor_add(
                                dsum[:, 0:1], dsum[:, 0:1], den2_kt0[:]
                            )
                        rec = smalls.tile([128, 2], F32, tag="rec")
                        nc.vector.reciprocal(rec[:], dsum[:])
                        vs = vsp.tile([128, 2, DH], BF16, tag="vs")
                        vss[kt_] = vs
                        for hh in range(2):
                            nc.gpsimd.tensor_scalar_mul(
                                vs[:, hh, :],
                                in0=v_sb[
                                    :, kt_, (2 * p + hh) * 64 : (2 * p + hh + 1) * 64
                                ],
                                scalar1=rec[:, hh : hh + 1],
                            )

                for kt in range(4 if p == 0 else 0, KT):
                    ex = expp.tile([128, 2, S], BF16, tag="exp")
                    exs[kt] = ex
                    den = smalls.tile([128, 2, 2], F32, tag="den")
                    den2 = None
                    for half in range(2):
                        for hh in range(2):
                            sp = sp_ps.tile([128, 1024], F32, tag="sp")
                            for qc in range(2):
                                q0 = half * 1024 + qc * 512
                                nc.tensor.matmul(
                                    sp[:, qc * 512 : (qc + 1) * 512],
                                    qk_sb[
                                        hh * 64 : (hh + 1) * 64,
                                        2 + p,
                                        kt * 128 : (kt + 1) * 128,
                                    ],
                                    qk_sb[hh * 64 : (hh + 1) * 64, p, q0 : q0 + 512],
                                    start=True,
                                    stop=True,
                                )
                            exc = ex[:, hh, half * 1024 : (half + 1) * 1024]
                            dslice = den[:, hh, half : half + 1]
                            # offload softmax-denominator accumulation to DVE
                            # for ~40 of each pair's 64 chunks (ACT/DVE
                            # balance); keep the final k-tile on ACT so the
                            # tail's first quad isn't gated on a DVE reduce
                            # denominators via DVE tensor_scalar(mult 1.0)
                            # with fused accum: 4x DVE mode makes this ~330ns
                            # so ACT keeps only the exps. Final k-tile stays
                            # on ACT accum (shortest path into the tail).
                            offload = True
                            if offload:
                                nc.scalar.activation(exc, sp[:], AF.Exp, scale=0.125)
                                nc.vector.tensor_scalar(
                                    out=exc,
                                    in0=exc,
                                    scalar1=1.0,
                                    scalar2=None,
                                    op0=mybir.AluOpType.mult,
                                    op1=mybir.AluOpType.add,
                                    accum_out=dslice,
                                )
                            else:
                                nc.scalar.activation(
                                    exc, sp[:], AF.Exp, scale=0.125, accum_out=dslice
                                )
                        if kt == 0 and half == 0 and hh == 1 and p == 0:
                            # Q cols 1024-2047 for pair0 (needed by half1);
                            # pair1's Q groups all complete during pair0.
                            emit_qk_group(0, 2)
                            emit_qk_group(0, 3)

                    # attn.V quad of the previous group (or pair0 spill)
                    if kt >= FG:
                        g, o = kt // FG - 1, kt % FG
                        emit_quad(p, g, o, exs, vss)
                    elif p == 1:
                        emit_quad(0, 3, kt, exs_p0, vss_p0)

                    # fillers / pair0-tail / proj interleaves
                    if p == 0:
                        for _ in range(pops0[kt]):
                            if fillers0:
                                fillers0.pop()()
                    else:
                        for _ in range(pops1[kt]):
                            if fillers1:
                                fillers1.pop()()
                        if 2 <= kt <= 5:
                            emit_tblock(0, kt - 2)
                        if kt >= 3:
                            for _ in range(proj0[kt - 3]):
                                if proj_done < NQUAD * (kt - 1):
                                    emit_d_group(0, proj_done, tail=False)
                                    proj_done += 1

                    # denominators -> 1/denom -> scaled V for this k-tile
                    # (combine/scale on Pool; reciprocal is DVE-only)
                    # final k-tile: the whole chain stays on DVE (no
                    # cross-engine hops on the path into the tail)
                    last = p == 1 and kt == KT - 1
                    eng = nc.vector if last else nc.gpsimd
                    dsum = smalls.tile([128, 2], F32, tag="dsum")
                    eng.tensor_add(dsum[:], den[:, :, 0], den[:, :, 1])
                    if den2 is not None:
                        eng.tensor_add(dsum[:, 0:1], dsum[:, 0:1], den2[:])
                    rec = smalls.tile([128, 2], F32, tag="rec")
                    nc.vector.reciprocal(rec[:], dsum[:])
                    vs = vsp.tile([128, 2, DH], BF16, tag="vs")
                    vss[kt] = vs
                    for hh in range(2):
                        eng.tensor_scalar_mul(
                            vs[:, hh, :],
                            in0=v_sb[:, kt, (2 * p + hh) * 64 : (2 * p + hh + 1) * 64],
                            scalar1=rec[:, hh : hh + 1],
                        )

                if p == 0:
                    exs_p0, vss_p0 = exs, vss
                else:
                    # tail: stay one quad ahead so PE never idles while a
                    # block's drain/convert hop across DVE/ACT
                    emit_quad(1, 3, 0, exs, vss)
                    for o in range(NQUAD):
                        emit_tblock(1, o, tail=True)
                        if o + 1 < NQUAD:
                            emit_quad(1, 3, o + 1, exs, vss)
                        for st in range(NQUAD * o, NQUAD * (o + 1)):
                            emit_d_group(1, st, tail=True)

    nc.compile()
    return nc


def _shard_inputs(input, Wqkv, bqkv, Wo):
    """Build the 8 per-core input dicts (host-side layout/sharding)."""
    bf16 = ml_dtypes.bfloat16
    ident_f32 = np.eye(128, dtype=np.float32)
    in_maps = []
    for c in range(NCORES):
        b = c // 4
        g = c % 4
        heads = range(4 * g, 4 * g + 4)
        rows = (
            [slice(64 * h, 64 * h + 64) for h in heads]
            + [slice(E + 64 * h, E + 64 * h + 64) for h in heads]
            + [slice(2 * E + 64 * h, 2 * E + 64 * h + 64) for h in heads]
        )
        W_sel = np.concatenate([Wqkv[s] for s in rows], axis=0)  # [768, 1024]
        b_sel = np.concatenate([bqkv[s] for s in rows], axis=0)  # [768]
        in_maps.append(
            {
                "xT": np.ascontiguousarray(input[b].T).astype(bf16),
                "wT": np.ascontiguousarray(W_sel.T).astype(bf16),
                "bq": np.ascontiguousarray(b_sel[:QK].reshape(4, 128).T),
                "bv": np.ascontiguousarray(b_sel[QK:V3].reshape(1, 256)).astype(bf16),
                "woT": np.ascontiguousarray(
                    Wo[:, 4 * g * DH : 4 * (g + 1) * DH].T
                ).astype(bf16),
                "ident": ident_f32,
            }
        )
    return in_maps


def kernel(input, Wqkv, bqkv, Wo, bo, _trace=False):
    global LAST_RESULTS
    input = np.asarray(input, dtype=np.float32)
    Wqkv = np.asarray(Wqkv, dtype=np.float32)
    bqkv = np.asarray(bqkv, dtype=np.float32)
    Wo = np.asarray(Wo, dtype=np.float32)
    bo = np.asarray(bo, dtype=np.float32)

    nc = build_kernel()
    in_maps = _shard_inputs(input, Wqkv, bqkv, Wo)
    kwargs = {}
    if _trace:
        kwargs = dict(trace=True, trace_cores=[0])
    try:
        res = run_bass_kernel_spmd(nc, in_maps, core_ids=list(range(NCORES)), **kwargs)
    except ModuleNotFoundError:
        # no NTFF profiling hook in this container — run without trace
        res = run_bass_kernel_spmd(nc, in_maps, core_ids=list(range(NCORES)))
    LAST_RESULTS = res

    out = np.zeros((B, S, E), dtype=np.float32)
    for c in range(NCORES):
        out[c // 4] += res.results[c]["out0"]
        out[c // 4] += res.results[c]["out1"]
    out += bo
    return out
